# revision 1
# baseline (speedup 1.0000x reference)
"""Trainium2 Bass kernel: LookupTransformerBlock (block-causal sparse attention).

Reference semantics (B=4, T=784, D=768, H=12, Dh=64, d_ff=3072):
  x_aug = LN1(concat(memory[:, :T], x))              # [B, 2T, D], ln1 g=1/b=0
  h     = LN_att(x_aug)
  qkv   = h @ w_qkv.T ; block-causal attention over frames of 196
  x2    = x_aug + attn_out
  out   = (x2 + FFN(LN2(x2)))[:, T:, :]

Sharding: 8 cores = (batch b in 0..3) x (query-half hf in 0..1); each core
computes its 392 output rows with K/V over all 1568 positions (data-parallel,
no collectives).  All cores run one SPMD program; per-core differences (query
slice, attention mask extents) are carried in the input data, never in code.

Host-side preprocessing (layout/constant folds only, no activation math):
  - concat + transpose of inputs to feature-major x_aug^T
  - weight transposes; LN_att gains and softmax scale folded into w_qkv;
    LN2 gains folded into w1; K-bias dropped (softmax shift invariance);
    V-bias folded into b_out via softmax row-sum identity.

On-device pipeline (feature-major activations; PE contracts over partitions):
  LN stats via ones-matmul column sums + elementwise x^2, per-token scale
  broadcast via DRAM-bounce DMA; QKV GEMMs; scores^T per (head, j-tile) with
  mask applied as per-partition scale/bias on the Exp activation; PV with a
  ones-column appended to V so softmax denominators fall out of the same
  matmul; out-projection + residual; LN2; interleaved FFN1(silu)/FFN2; final
  PE transposes to token-major output.
"""

import os
import sys
from contextlib import ExitStack

import numpy as np

for _p in ("/opt/trn_rl_repo", os.path.expanduser("~/.axon_site/_ro/trn_rl_repo")):
    if os.path.isdir(_p) and _p not in sys.path:
        sys.path.append(_p)

import concourse.bass as bass
import concourse.bacc as bacc
import concourse.mybir as mybir
import concourse.tile as tile
from concourse.bass_utils import run_bass_kernel_spmd
from concourse.masks import make_identity

F32 = mybir.dt.float32
F32R = mybir.dt.float32r
AF = mybir.ActivationFunctionType
ALU = mybir.AluOpType

B = 4
T = 784
D = 768
L = 2 * T            # 1568
NQ = 392             # query rows per core
H = 12
DH = 64
DFF = 3072
NPATCH = 196
DC = D // 128        # 6
FT = DFF // 128      # 24
NJT = 13             # j-tiles over L (12 x 128 + 32)
JSZ = [128] * 12 + [32]
LCH = [512, 512, 512, 32]
EPS = 1e-5
NCORES = 8
JLO = 7              # first j-tile that can contain the frame-A mask boundary


def _stats_and_rows(nc, pmm, prow, psq, ones, eps1, xtiles, lch, want_rs1):
    """Column stats over D for feature-major tiles xtiles (6 x [128, lch]).

    Returns SBUF rows (mu, S, rs1?) where S = rs1*rs2 is the fused
    LN1+LN_att scale (rs2 from renormalizing LN1's output).  If want_rs1 is
    False (single LN), S = rs1 and no separate rs1 row is returned.
    """
    mu_ps = pmm.tile([1, lch], F32, tag="mm", name="mu_ps")
    msq_ps = pmm.tile([1, lch], F32, tag="mm", name="msq_ps")
    for dc in range(DC):
        nc.tensor.matmul(mu_ps[:], lhsT=ones[:], rhs=xtiles[dc][:, 0:lch],
                         start=(dc == 0), stop=(dc == DC - 1))
    for dc in range(DC):
        sq = psq.tile([128, lch], F32, tag="sq")
        nc.scalar.activation(sq[:], xtiles[dc][:, 0:lch], AF.Square)
        nc.tensor.matmul(msq_ps[:], lhsT=ones[:], rhs=sq[:],
                         start=(dc == 0), stop=(dc == DC - 1))
    r_mu = prow.tile([1, lch], F32, tag="row")
    nc.vector.tensor_copy(r_mu[:], mu_ps[:])
    r_var = prow.tile([1, lch], F32, tag="row")
    nc.vector.tensor_mul(r_var[:], r_mu[:], r_mu[:])
    nc.vector.tensor_sub(r_var[:], msq_ps[:], r_var[:])
    r_rs1 = prow.tile([1, lch], F32, tag="row")
    nc.scalar.activation(r_rs1[:], r_var[:], AF.Sqrt, bias=eps1[0:1, 0:1])
    nc.vector.reciprocal(r_rs1[:], r_rs1[:])
    if not want_rs1:
        return r_mu, r_rs1, None
    r_S = prow.tile([1, lch], F32, tag="row")
    nc.vector.tensor_mul(r_S[:], r_rs1[:], r_rs1[:])
    nc.vector.tensor_mul(r_S[:], r_var[:], r_S[:])          # var2 = var*rs1^2
    nc.scalar.activation(r_S[:], r_S[:], AF.Sqrt, bias=eps1[0:1, 0:1])
    nc.vector.reciprocal(r_S[:], r_S[:])                    # rs2
    nc.vector.tensor_mul(r_S[:], r_rs1[:], r_S[:])          # S = rs1*rs2
    return r_mu, r_S, r_rs1


def _phase_ab(nc, tc, ctx, env):
    """LN1+LN_att fused normalization, then K^T, Q^T, V GEMMs."""
    xT, xqT, wqkvT, scr = env["xT"], env["xqT"], env["wqkvT"], env["scr"]
    ones, cbq_sb = env["ones"], env["cbq_sb"]
    KT, QT, VA, y1T = env["KT"], env["QT"], env["VA"], env["y1T"]

    px = ctx.enter_context(tc.tile_pool(name="ab_x", bufs=7))
    psq = ctx.enter_context(tc.tile_pool(name="ab_sq", bufs=2))
    ptmp = ctx.enter_context(tc.tile_pool(name="ab_tmp", bufs=2))
    prow = ctx.enter_context(tc.tile_pool(name="ab_rows", bufs=5))
    pbc = ctx.enter_context(tc.tile_pool(name="ab_bc", bufs=3))
    pnt = ctx.enter_context(tc.tile_pool(name="ab_nt", bufs=DC))
    pnq = ctx.enter_context(tc.tile_pool(name="ab_nq", bufs=DC))
    pw = ctx.enter_context(tc.tile_pool(name="ab_w", bufs=4))
    pwv = ctx.enter_context(tc.tile_pool(name="ab_wv", bufs=2))
    pmm = ctx.enter_context(tc.tile_pool(name="ab_mm", bufs=4, space="PSUM"))
    ppsv = ctx.enter_context(tc.tile_pool(name="ab_psv", bufs=2, space="PSUM"))

    nT = [pnt.tile([128, L], F32R, tag="nt", name=f"nT{i}") for i in range(DC)]

    # LN1 + LN_att fused, per l-chunk (feature-major)
    for ci in range(4):
        lch = LCH[ci]
        l0 = ci * 512
        xc = []
        for dc in range(DC):
            t = px.tile([128, lch], F32, tag="xc", name="xc")
            nc.sync.dma_start(t[:], xT[dc * 128:(dc + 1) * 128, l0:l0 + lch])
            xc.append(t)
        r_mu, r_S, _ = _stats_and_rows(nc, pmm, prow, psq, ones, env["eps1"], xc, lch, True)
        nc.sync.dma_start(scr[ci:ci + 1, 0:lch], r_mu[:])
        nc.sync.dma_start(scr[4 + ci:5 + ci, 0:lch], r_S[:])
        mu_b = pbc.tile([128, lch], F32, tag="bc")
        nc.sync.dma_start(mu_b[:], scr[ci:ci + 1, 0:lch].to_broadcast((128, lch)))
        S_b = pbc.tile([128, lch], F32, tag="bc")
        nc.sync.dma_start(S_b[:], scr[4 + ci:5 + ci, 0:lch].to_broadcast((128, lch)))
        for dc in range(DC):
            tmp = ptmp.tile([128, lch], F32, tag="tmpa")
            nc.vector.tensor_sub(tmp[:], xc[dc][:], mu_b[:])
            nc.vector.tensor_mul(nT[dc][:, l0:l0 + lch], tmp[:], S_b[:])

    # q-slice stats (n^T and y1^T for the 392 query columns)
    nqT = [pnq.tile([128, NQ], F32R, tag="nq", name=f"nqT{i}") for i in range(DC)]
    xq = []
    for dc in range(DC):
        t = px.tile([128, NQ], F32, tag="xc", name="xq")
        nc.sync.dma_start(t[:], xqT[dc * 128:(dc + 1) * 128, :])
        xq.append(t)
    r_muq, r_Sq, r_rs1q = _stats_and_rows(nc, pmm, prow, psq, ones, env["eps1"], xq, NQ, True)
    nc.sync.dma_start(scr[8:9, 0:NQ], r_muq[:])
    nc.sync.dma_start(scr[9:10, 0:NQ], r_Sq[:])
    nc.sync.dma_start(scr[10:11, 0:NQ], r_rs1q[:])
    mu_qb = pbc.tile([128, NQ], F32, tag="bc")
    nc.sync.dma_start(mu_qb[:], scr[8:9, 0:NQ].to_broadcast((128, NQ)))
    S_qb = pbc.tile([128, NQ], F32, tag="bc")
    nc.sync.dma_start(S_qb[:], scr[9:10, 0:NQ].to_broadcast((128, NQ)))
    rs1_qb = pbc.tile([128, NQ], F32, tag="bc")
    nc.sync.dma_start(rs1_qb[:], scr[10:11, 0:NQ].to_broadcast((128, NQ)))
    for dc in range(DC):
        tmp = ptmp.tile([128, NQ], F32, tag="tmpa")
        nc.vector.tensor_sub(tmp[:], xq[dc][:], mu_qb[:])
        nc.vector.tensor_mul(nqT[dc][:], tmp[:], S_qb[:])
        nc.vector.tensor_mul(y1T[dc][:], tmp[:], rs1_qb[:])

    # K^T  (e-tiles 6..11 of qkv)
    for et in range(DC):
        ps_k = [pmm.tile([128, LCH[ci]], F32, tag="mm", name=f"ps_k{ci}") for ci in range(4)]
        for dc in range(DC):
            wkt = pw.tile([128, 128], F32R, tag="w128")
            nc.sync.dma_start(
                wkt[:], wqkvT[dc * 128:(dc + 1) * 128, D + et * 128:D + (et + 1) * 128])
            for ci in range(4):
                nc.tensor.matmul(ps_k[ci][:], lhsT=wkt[:],
                                 rhs=nT[dc][:, ci * 512:ci * 512 + LCH[ci]],
                                 start=(dc == 0), stop=(dc == DC - 1))
        for ci in range(4):
            nc.vector.tensor_copy(KT[et][:, ci * 512:ci * 512 + LCH[ci]], ps_k[ci][:])

    # Q^T (e-tiles 0..5) with folded bias
    for et in range(DC):
        ps_q = pmm.tile([128, NQ], F32, tag="mm")
        for dc in range(DC):
            wqt = pw.tile([128, 128], F32R, tag="w128")
            nc.sync.dma_start(
                wqt[:], wqkvT[dc * 128:(dc + 1) * 128, et * 128:(et + 1) * 128])
            nc.tensor.matmul(ps_q[:], lhsT=wqt[:], rhs=nqT[dc][:],
                             start=(dc == 0), stop=(dc == DC - 1))
        nc.scalar.activation(QT[et][:], ps_q[:], AF.Identity, bias=cbq_sb[:, et:et + 1])

    # V token-major, ones column appended per head
    for lt2 in range((NJT + 1) // 2):
        wv = []
        for dc in range(DC):
            t = pwv.tile([128, D], F32R, tag="wv", name="wv")
            nc.sync.dma_start(t[:], wqkvT[dc * 128:(dc + 1) * 128, 2 * D:3 * D])
            wv.append(t)
        for lt in (2 * lt2, 2 * lt2 + 1):
            if lt >= NJT:
                continue
            lsz = JSZ[lt]
            ps_v = ppsv.tile([128, D], F32, tag="psv")
            for dc in range(DC):
                lhsT = nT[dc][:, lt * 128:lt * 128 + lsz]
                nc.tensor.matmul(ps_v[0:lsz, 0:512], lhsT=lhsT, rhs=wv[dc][:, 0:512],
                                 start=(dc == 0), stop=(dc == DC - 1),
                                 skip_group_check=True)
                nc.tensor.matmul(ps_v[0:lsz, 512:D], lhsT=lhsT, rhs=wv[dc][:, 512:D],
                                 start=(dc == 0), stop=(dc == DC - 1),
                                 skip_group_check=True)
            vav = VA[lt][:].rearrange("p (h c) -> p h c", c=65)
            nc.sync.dma_start(vav[:, :, 64:65],
                              env["vones"][:].to_broadcast((128, 12, 1)))
            nc.vector.tensor_copy(vav[0:lsz, :, 0:64],
                                  ps_v[0:lsz, :].rearrange("p (h c) -> p h c", c=64))


def _phase_attn(nc, tc, ctx, env):
    """Scores^T, masked exp, PV (with softmax sums via the ones column),
    per-head normalization into feature-major ONT."""
    KT, QT, VA, ONT = env["KT"], env["QT"], env["VA"], env["ONT"]
    msk_sb, scr = env["msk_sb"], env["scr"]

    ppt = ctx.enter_context(tc.tile_pool(name="c_pt", bufs=3))
    prb = ctx.enter_context(tc.tile_pool(name="c_rb", bufs=2))
    pot = ctx.enter_context(tc.tile_pool(name="c_ot", bufs=2))
    prow2 = ctx.enter_context(tc.tile_pool(name="c_rows", bufs=2))
    pss = ctx.enter_context(tc.tile_pool(name="c_ps_s", bufs=3, space="PSUM"))
    pso = ctx.enter_context(tc.tile_pool(name="c_ps_o", bufs=2, space="PSUM"))

    for hp in range(6):
        o_ps = [pso.tile([65, NQ], F32, tag="pso", name=f"o_ps{i}") for i in range(2)]
        for jt in range(NJT):
            jsz = JSZ[jt]
            for hi in range(2):
                h = 2 * hp + hi
                part = 64 * hi
                s_ps = pss.tile([128, NQ], F32, tag="ps_s")
                nc.tensor.matmul(
                    s_ps[0:jsz, :],
                    lhsT=KT[hp][part:part + 64, jt * 128:jt * 128 + jsz],
                    rhs=QT[hp][part:part + 64, :], start=True, stop=True)
                pt = ppt.tile([128, NQ], F32R, tag="pt")
                nc.scalar.activation(
                    pt[0:jsz, :], s_ps[0:jsz, :], AF.Exp,
                    bias=msk_sb[0:jsz, NJT + jt:NJT + jt + 1],
                    scale=msk_sb[0:jsz, jt:jt + 1])
                if jt >= JLO:
                    nc.scalar.activation(
                        pt[0:jsz, 0:NPATCH], s_ps[0:jsz, 0:NPATCH], AF.Exp,
                        bias=msk_sb[0:jsz, 3 * NJT + jt:3 * NJT + jt + 1],
                        scale=msk_sb[0:jsz, 2 * NJT + jt:2 * NJT + jt + 1])
                nc.tensor.matmul(
                    o_ps[hi][:], lhsT=VA[jt][0:jsz, h * 65:(h + 1) * 65],
                    rhs=pt[0:jsz, :], start=(jt == 0), stop=(jt == NJT - 1),
                    skip_group_check=True)
        for hi in range(2):
            h = 2 * hp + hi
            part = 64 * hi
            rcp = prow2.tile([1, NQ], F32, tag="rrow")
            nc.vector.reciprocal(rcp[:], o_ps[hi][64:65, :])
            nc.sync.dma_start(scr[16 + h:17 + h, 0:NQ], rcp[:])
            rb = prb.tile([64, NQ], F32, tag="rb")
            nc.sync.dma_start(rb[:], scr[16 + h:17 + h, 0:NQ].to_broadcast((64, NQ)))
            ot = pot.tile([64, NQ], F32R, tag="otmp")
            nc.vector.tensor_mul(ot[:], o_ps[hi][0:64, :], rb[:])
            nc.sync.dma_start(ONT[hp][part:part + 64, :], ot[:])


def _phase_outproj(nc, tc, ctx, env):
    woutT, ONT, y1T, x2T, bout_sb = (
        env["woutT"], env["ONT"], env["y1T"], env["x2T"], env["bout_sb"])
    pwD = ctx.enter_context(tc.tile_pool(name="d_w", bufs=4))
    pmmD = ctx.enter_context(tc.tile_pool(name="d_mm", bufs=2, space="PSUM"))
    for dt in range(DC):
        ps = pmmD.tile([128, NQ], F32, tag="mmD")
        for et in range(DC):
            wt = pwD.tile([128, 128], F32R, tag="wD")
            nc.sync.dma_start(
                wt[:], woutT[et * 128:(et + 1) * 128, dt * 128:(dt + 1) * 128])
            nc.tensor.matmul(ps[:], lhsT=wt[:], rhs=ONT[et][:],
                             start=(et == 0), stop=(et == DC - 1))
        nc.vector.scalar_tensor_tensor(
            x2T[dt][:], ps[:], env["bout_sb"][:, dt:dt + 1], y1T[dt][:],
            op0=ALU.add, op1=ALU.add)


def _phase_ffn(nc, tc, ctx, env):
    """LN2 + interleaved FFN1(silu)/FFN2 with residual."""
    w1T, w2T, scr = env["w1T"], env["w2T"], env["scr"]
    ones, cb1_sb, b2_sb = env["ones"], env["cb1_sb"], env["b2_sb"]
    x2T, outT = env["x2T"], env["outT"]

    psq2 = ctx.enter_context(tc.tile_pool(name="e_sq", bufs=3))
    prow3 = ctx.enter_context(tc.tile_pool(name="e_rows", bufs=5))
    pbc2 = ctx.enter_context(tc.tile_pool(name="e_bc", bufs=2))
    pn2 = ctx.enter_context(tc.tile_pool(name="e_n2", bufs=DC))
    pwF = ctx.enter_context(tc.tile_pool(name="f_w", bufs=6))
    pffs = ctx.enter_context(tc.tile_pool(name="f_ffs", bufs=3))
    pmmE = ctx.enter_context(tc.tile_pool(name="ef_mm", bufs=2, space="PSUM"))
    pacc = ctx.enter_context(tc.tile_pool(name="f_acc", bufs=DC, space="PSUM"))

    r_mu2, r_S2, _ = _stats_and_rows(nc, pmmE, prow3, psq2, ones, env["eps1"], x2T, NQ, False)
    nc.sync.dma_start(scr[12:13, 0:NQ], r_mu2[:])
    nc.sync.dma_start(scr[13:14, 0:NQ], r_S2[:])
    mu2_b = pbc2.tile([128, NQ], F32, tag="bc2")
    nc.sync.dma_start(mu2_b[:], scr[12:13, 0:NQ].to_broadcast((128, NQ)))
    S2_b = pbc2.tile([128, NQ], F32, tag="bc2")
    nc.sync.dma_start(S2_b[:], scr[13:14, 0:NQ].to_broadcast((128, NQ)))
    n2T = []
    for dc in range(DC):
        t = pn2.tile([128, NQ], F32R, tag="n2", name="n2")
        tmp = psq2.tile([128, NQ], F32, tag="sq")
        nc.vector.tensor_sub(tmp[:], x2T[dc][:], mu2_b[:])
        nc.vector.tensor_mul(t[:], tmp[:], S2_b[:])
        n2T.append(t)

    ps_acc = [pacc.tile([128, NQ], F32, tag="acc", name=f"ps_acc{i}") for i in range(DC)]
    for ft in range(FT):
        ps1 = pmmE.tile([128, NQ], F32, tag="mm")
        for dc in range(DC):
            w1t = pwF.tile([128, 128], F32R, tag="wF")
            nc.sync.dma_start(
                w1t[:], w1T[dc * 128:(dc + 1) * 128, ft * 128:(ft + 1) * 128])
            nc.tensor.matmul(ps1[:], lhsT=w1t[:], rhs=n2T[dc][:],
                             start=(dc == 0), stop=(dc == DC - 1))
        # silu(u) = u * sigmoid(u) with u = ps1 + cb1 (CoreSim lacks Silu)
        sig = pffs.tile([128, NQ], F32, tag="sig")
        nc.scalar.activation(sig[:], ps1[:], AF.Sigmoid, bias=cb1_sb[:, ft:ft + 1])
        ffs = pffs.tile([128, NQ], F32R, tag="ffs")
        nc.vector.scalar_tensor_tensor(ffs[:], ps1[:], cb1_sb[:, ft:ft + 1], sig[:],
                                       op0=ALU.add, op1=ALU.mult)
        for dt in range(DC):
            w2t = pwF.tile([128, 128], F32R, tag="wF")
            nc.sync.dma_start(
                w2t[:], w2T[ft * 128:(ft + 1) * 128, dt * 128:(dt + 1) * 128])
            nc.tensor.matmul(ps_acc[dt][:], lhsT=w2t[:], rhs=ffs[:],
                             start=(ft == 0), stop=(ft == FT - 1),
                             skip_group_check=True)
    for dt in range(DC):
        nc.vector.scalar_tensor_tensor(
            outT[dt][:], ps_acc[dt][:], b2_sb[:, dt:dt + 1], x2T[dt][:],
            op0=ALU.add, op1=ALU.add)


def _phase_store(nc, tc, ctx, env):
    """Transpose feature-major result to token-major and store."""
    outT, ident, out = env["outT"], env["ident"], env["out"]
    posb = ctx.enter_context(tc.tile_pool(name="h_osb", bufs=2))
    ptr = ctx.enter_context(tc.tile_pool(name="h_tr", bufs=2, space="PSUM"))
    QSZ = [128, 128, 128, 8]
    for qt in range(4):
        qsz = QSZ[qt]
        osb = posb.tile([128, D], F32, tag="osb")
        for dt in range(DC):
            tp = ptr.tile([128, 128], F32, tag="ptr")
            nc.tensor.transpose(tp[0:qsz, :],
                                outT[dt][:, qt * 128:qt * 128 + qsz], ident[:])
            nc.scalar.copy(osb[0:qsz, dt * 128:(dt + 1) * 128], tp[0:qsz, :])
        nc.sync.dma_start(out[qt * 128:qt * 128 + qsz, :], osb[0:qsz, :])


def build_program():
    nc = bacc.Bacc("TRN2")
    env = {}
    env["xT"] = nc.declare_dram_parameter("xT", [D, L], F32, isOutput=False)
    env["xqT"] = nc.declare_dram_parameter("xqT", [D, NQ], F32, isOutput=False)
    env["wqkvT"] = nc.declare_dram_parameter("wqkvT", [D, 3 * D], F32R, isOutput=False)
    cbq = nc.declare_dram_parameter("cbq", [128, DC], F32, isOutput=False)
    env["woutT"] = nc.declare_dram_parameter("woutT", [D, D], F32R, isOutput=False)
    bout = nc.declare_dram_parameter("bout", [128, DC], F32, isOutput=False)
    env["w1T"] = nc.declare_dram_parameter("w1T", [D, DFF], F32R, isOutput=False)
    cb1 = nc.declare_dram_parameter("cb1", [128, FT], F32, isOutput=False)
    env["w2T"] = nc.declare_dram_parameter("w2T", [DFF, D], F32R, isOutput=False)
    b2 = nc.declare_dram_parameter("b2", [128, DC], F32, isOutput=False)
    msk = nc.declare_dram_parameter("msk", [128, 4 * NJT], F32, isOutput=False)
    env["out"] = nc.declare_dram_parameter("out", [NQ, D], F32, isOutput=True)
    env["vones"] = nc.declare_dram_parameter("vones", [128, 1], F32R, isOutput=False)
    env["scr"] = nc.dram_tensor("scr", [32, 512], F32)

    with tile.TileContext(nc) as tc, ExitStack() as top:
        pc = top.enter_context(tc.tile_pool(name="const", bufs=1))
        px2 = top.enter_context(tc.tile_pool(name="x2p", bufs=DC))
        poutT = top.enter_context(tc.tile_pool(name="outTp", bufs=DC))

        ones = pc.tile([128, 1], F32, tag="ones")
        nc.vector.memset(ones[:], 1.0 / D)
        eps1 = pc.tile([1, 1], F32, tag="eps1")
        nc.vector.memset(eps1[:], EPS)
        env["eps1"] = eps1
        ident = pc.tile([128, 128], F32, tag="ident")
        make_identity(nc, ident[:])
        env["ones"], env["ident"] = ones, ident
        for name, prm, w in (("cbq_sb", cbq, DC), ("bout_sb", bout, DC),
                             ("b2_sb", b2, DC), ("cb1_sb", cb1, FT),
                             ("msk_sb", msk, 4 * NJT)):
            t = pc.tile([128, w], F32, tag=name, name=name)
            nc.sync.dma_start(t[:], prm[:])
            env[name] = t

        env["x2T"] = [px2.tile([128, NQ], F32, tag="x2", name=f"x2T{i}") for i in range(DC)]
        env["outT"] = [poutT.tile([128, NQ], F32, tag="outT", name=f"outT{i}") for i in range(DC)]

        with ExitStack() as mid:
            pkt = mid.enter_context(tc.tile_pool(name="ktp", bufs=DC))
            pqt = mid.enter_context(tc.tile_pool(name="qtp", bufs=DC))
            pva = mid.enter_context(tc.tile_pool(name="vap", bufs=NJT))
            py1 = mid.enter_context(tc.tile_pool(name="y1p", bufs=DC))
            env["KT"] = [pkt.tile([128, L], F32R, tag="kt", name=f"KT{i}") for i in range(DC)]
            env["QT"] = [pqt.tile([128, NQ], F32R, tag="qt", name=f"QT{i}") for i in range(DC)]
            env["VA"] = [pva.tile([128, 12 * 65], F32R, tag="va", name=f"VA{i}") for i in range(NJT)]
            env["y1T"] = [py1.tile([128, NQ], F32, tag="y1", name=f"y1T{i}") for i in range(DC)]

            with ExitStack() as ctx:
                _phase_ab(nc, tc, ctx, env)

            with ExitStack() as ctx:
                pont = ctx.enter_context(tc.tile_pool(name="ontp", bufs=DC))
                env["ONT"] = [pont.tile([128, NQ], F32R, tag="ont", name=f"ONT{i}") for i in range(DC)]
                with ExitStack() as inner:
                    _phase_attn(nc, tc, inner, env)
                with ExitStack() as inner:
                    _phase_outproj(nc, tc, inner, env)

        with ExitStack() as ctx:
            _phase_ffn(nc, tc, ctx, env)
        with ExitStack() as ctx:
            _phase_store(nc, tc, ctx, env)

    nc.finalize()
    return nc


_NC = None


def _get_nc():
    global _NC
    if _NC is None:
        _NC = build_program()
    return _NC


def _host_prepare(inputs):
    """Fold constants and lay out per-core input maps."""
    f32 = np.float32
    x = np.asarray(inputs["x"], f32)
    memory = np.asarray(inputs["memory"], f32)
    w_qkv = np.asarray(inputs["w_qkv"], f32)
    w_out = np.asarray(inputs["w_out"], f32)
    b_out = np.asarray(inputs["b_out"], f32)
    g_att = np.asarray(inputs["ln_att_g"], f32)
    b_att = np.asarray(inputs["ln_att_b"], f32)
    g2 = np.asarray(inputs["ln2_g"], f32)
    bb2 = np.asarray(inputs["ln2_b"], f32)
    w1 = np.asarray(inputs["w1"], f32)
    b1 = np.asarray(inputs["b1"], f32)
    w2 = np.asarray(inputs["w2"], f32)
    b2v = np.asarray(inputs["b2"], f32)

    qscale = f32(DH ** -0.5)
    w_qkv_eff = w_qkv * g_att[None, :]
    w_qkv_eff[:D] *= qscale
    cb_qkv = w_qkv @ b_att
    cb_q = (cb_qkv[:D] * qscale).astype(f32)
    cb_v = cb_qkv[2 * D:].astype(f32)
    b_out_eff = (b_out + w_out @ cb_v).astype(f32)
    w1_eff = w1 * g2[None, :]
    cb1_eff = (w1 @ bb2 + b1).astype(f32)

    def cols(v):
        # [N] vector -> [128, N//128] per-partition bias layout
        return np.ascontiguousarray(v.reshape(-1, 128).T)

    shared = {
        "wqkvT": np.ascontiguousarray(w_qkv_eff.T),
        "cbq": cols(cb_q),
        "woutT": np.ascontiguousarray(w_out.T),
        "bout": cols(b_out_eff),
        "w1T": np.ascontiguousarray(w1_eff.T),
        "cb1": cols(cb1_eff),
        "w2T": np.ascontiguousarray(w2.T),
        "b2": cols(b2v),
    }

    in_maps = []
    for c in range(NCORES):
        b, hf = divmod(c, 2)
        x_aug = np.concatenate([memory[b, :T], x[b]], axis=0)      # [L, D]
        q0 = T + hf * NQ
        LcA = (5 + 2 * hf) * NPATCH
        LcB = (6 + 2 * hf) * NPATCH
        j = np.arange(NJT * 128)
        sa = ((j < LcB) & (j < L)).astype(f32)
        ba = np.where(sa > 0, 0.0, -30.0).astype(f32)
        sq = (j < LcA).astype(f32)
        bq = np.where(sq > 0, 0.0, -30.0).astype(f32)
        mskv = np.concatenate(
            [v.reshape(NJT, 128).T for v in (sa, ba, sq, bq)], axis=1)
        in_maps.append({
            "xT": np.ascontiguousarray(x_aug.T),
            "xqT": np.ascontiguousarray(x_aug[q0:q0 + NQ].T),
            "msk": np.ascontiguousarray(mskv),
            "vones": np.ones((128, 1), f32),
            **shared,
        })
    return in_maps


def _assemble(results):
    out = np.zeros((B, T, D), np.float32)
    for c in range(NCORES):
        b, hf = divmod(c, 2)
        out[b, hf * NQ:(hf + 1) * NQ, :] = results[c]["out"]
    return out


def kernel(**inputs):
    nc = _get_nc()
    in_maps = _host_prepare(inputs)
    res = run_bass_kernel_spmd(nc, in_maps, list(range(NCORES)))
    return _assemble(res.results)


def _ensure_ntff_hook():
    """Provide antenv.axon_hooks (absent in this image) so trace=True can
    drive NTFF capture through libaxon_pjrt.so, mirroring trn_boot.py."""
    import contextlib
    import ctypes
    import types

    try:
        from antenv.axon_hooks import get_axon_ntff_profile_hook  # noqa: F401
        return
    except ImportError:
        pass
    import antenv

    so_path = "/opt/axon/libaxon_pjrt.so"
    lib = ctypes.CDLL(so_path)
    if not hasattr(lib, "axon_start_nrt_profile"):
        raise RuntimeError("libaxon_pjrt.so lacks NTFF profile symbols")
    lib.axon_start_nrt_profile.argtypes = [ctypes.POINTER(ctypes.c_int64),
                                           ctypes.c_size_t]
    lib.axon_start_nrt_profile.restype = ctypes.c_int64
    lib.axon_stop_nrt_profile.argtypes = [ctypes.c_char_p]
    lib.axon_stop_nrt_profile.restype = ctypes.c_int64

    @contextlib.contextmanager
    def _hook(output_dir, device_ids):
        import jax
        jax.devices()
        if device_ids:
            ids = (ctypes.c_int64 * len(device_ids))(*device_ids)
            rc = lib.axon_start_nrt_profile(ids, len(device_ids))
        else:
            rc = lib.axon_start_nrt_profile(None, 0)
        if rc != 0:
            raise RuntimeError(f"axon_start_nrt_profile rc={rc}")
        try:
            yield
        finally:
            n = lib.axon_stop_nrt_profile(str(output_dir).encode())
            print(f"ntff profile: {n} file(s) written to {output_dir}",
                  file=sys.stderr)

    box = {"h": _hook}
    mod = types.ModuleType("antenv.axon_hooks")
    mod.set_axon_ntff_profile_hook = lambda h: box.__setitem__("h", h)
    mod.get_axon_ntff_profile_hook = lambda: box["h"]
    sys.modules["antenv.axon_hooks"] = mod
    antenv.axon_hooks = mod


def kernel_traced(**inputs):
    """Like kernel() but with NTFF profiling; returns (out, exec_time_ns)."""
    import tempfile

    from concourse import bass_utils as _bu
    _ensure_ntff_hook()
    _bu.upload_artifacts = lambda tmpdir: f"local:{tmpdir}"  # no bucket creds here
    nc = _get_nc()
    in_maps = _host_prepare(inputs)
    tmpdir = tempfile.mkdtemp(prefix="ntff_")
    res = run_bass_kernel_spmd(nc, in_maps, list(range(NCORES)), trace=True,
                               tmpdir=tmpdir)
    return _assemble(res.results), res.exec_time_ns



# revision 13
# speedup vs baseline: 2.0183x; 2.0183x over previous
"""Trainium2 Bass kernel: LookupTransformerBlock (block-causal sparse attention).

Reference semantics (B=4, T=784, D=768, H=12, Dh=64, d_ff=3072):
  x_aug = LN1(concat(memory[:, :T], x))              # [B, 2T, D], ln1 g=1/b=0
  h     = LN_att(x_aug)
  qkv   = h @ w_qkv.T ; block-causal attention over frames of 196
  x2    = x_aug + attn_out
  out   = (x2 + FFN(LN2(x2)))[:, T:, :]

Sharding: 8 cores = (batch b in 0..3) x (query-half hf in 0..1); each core
computes its 392 output rows with K/V over all 1568 positions (data-parallel,
no collectives).  One SPMD program; per-core differences (query slice, mask
extents) live in input data only.

v2 design (vs the fp32r baseline):
  - all matmul operands bf16 (1-pass PE, half DMA), fp32 PSUM accumulation
  - no DRAM-bounce row broadcasts: rows are broadcast across partitions with
    tiny selector matmuls on the PE (ones-column outer products)
  - LN1 stats for all 4 L-chunks + the query slice accumulate into one
    [5, 392] PSUM pair via per-chunk selector columns; row math runs once
  - weights DMA'd in 6 big per-dc tiles each, prefetched at kernel start
  - frame-A mask boundary applied as a per-partition DVE multiply on the
    exp tile instead of a second Exp on the scalar engine
  - K/Q GEMMs for head-pair hp+1 are software-pipelined into the attention
    j-loop of head-pair hp so the PE never idles long enough to re-throttle
"""

import os
import sys
from contextlib import ExitStack

import numpy as np

for _p in ("/opt/trn_rl_repo", os.path.expanduser("~/.axon_site/_ro/trn_rl_repo")):
    if os.path.isdir(_p) and _p not in sys.path:
        sys.path.append(_p)

import concourse.bass as bass
import concourse.bacc as bacc
import concourse.mybir as mybir
import concourse.tile as tile
from concourse.bass_utils import run_bass_kernel_spmd
from concourse.masks import make_identity

F32 = mybir.dt.float32
BF = mybir.dt.bfloat16
AF = mybir.ActivationFunctionType
ALU = mybir.AluOpType

B = 4
T = 784
D = 768
L = 2 * T            # 1568
NQ = 392             # query rows per core
H = 12
DH = 64
DFF = 3072
NPATCH = 196
DC = D // 128        # 6
FT = DFF // 128      # 24
NJT = 13             # j-tiles over L (12 x 128 + 32)
JSZ = [128] * 12 + [32]
CH = 392             # LN1 l-chunk width (L = 4*CH)
NCH = 4
EPS = 1e-5
NCORES = 8
JLO = 7              # first j-tile that can contain a frame-A mask boundary


def _row_stats(nc, prow, eps5, mu_ps, msq_ps, nrow):
    """Row math on [nrow, CH] stats: returns (muR, SR, rs1R) bf16 rows.

    mu = sum/D;  var = sumsq/D - mu^2;  rs1 = 1/sqrt(var+eps);
    rs2 = 1/sqrt(var*rs1^2+eps);  S = rs1*rs2  (fused LN1+LN_att scale).
    """
    invD = 1.0 / D
    muR = prow.tile([nrow, CH], BF, tag="row", name="muR")
    nc.vector.tensor_scalar_mul(muR[:], mu_ps[:], invD)
    musq = prow.tile([nrow, CH], F32, tag="row", name="musq")
    nc.vector.tensor_mul(musq[:], muR[:], muR[:])
    var = prow.tile([nrow, CH], F32, tag="row", name="var")
    nc.vector.scalar_tensor_tensor(var[:], msq_ps[:], invD, musq[:],
                                   op0=ALU.mult, op1=ALU.subtract)
    sd = prow.tile([nrow, CH], F32, tag="row", name="sd")
    nc.scalar.activation(sd[:], var[:], AF.Sqrt, bias=eps5[0:nrow, 0:1])
    rs1R = prow.tile([nrow, CH], BF, tag="row", name="rs1R")
    nc.vector.reciprocal(rs1R[:], sd[:])
    v2 = prow.tile([nrow, CH], F32, tag="row", name="v2")
    nc.vector.tensor_mul(v2[:], rs1R[:], rs1R[:])
    nc.vector.tensor_mul(v2[:], var[:], v2[:])
    nc.scalar.activation(v2[:], v2[:], AF.Sqrt, bias=eps5[0:nrow, 0:1])
    rs2 = prow.tile([nrow, CH], F32, tag="row", name="rs2")
    nc.vector.reciprocal(rs2[:], v2[:])
    SR = prow.tile([nrow, CH], BF, tag="row", name="SR")
    nc.vector.tensor_mul(SR[:], rs1R[:], rs2[:])
    return muR, SR, rs1R


def _phase_ln1(nc, tc, ctx, env):
    """LN1+LN_att fused normalization of x_aug (4 chunks) and the q slice.

    Raw x is DMA'd straight into the nT/nqT tiles and normalized in place.
    """
    sel5, selbc, eps5 = env["sel5"], env["selbc"], env["eps5"]
    nT, nqT, y1T = env["nT"], env["nqT"], env["y1T"]

    psq = ctx.enter_context(tc.tile_pool(name="a_sq", bufs=6))
    ptm = ctx.enter_context(tc.tile_pool(name="a_tmp", bufs=4))
    prow = ctx.enter_context(tc.tile_pool(name="a_rows", bufs=8))
    pst = ctx.enter_context(tc.tile_pool(name="a_st", bufs=2, space="PSUM"))
    pbc = ctx.enter_context(tc.tile_pool(name="a_bc", bufs=4, space="PSUM"))

    xc, xq = nT, nqT

    mu_ps = pst.tile([5, CH], F32, tag="st", name="mu_ps")
    msq_ps = pst.tile([5, CH], F32, tag="st", name="msq_ps")
    for ci in range(NCH + 1):
        src = [xq[dc][:] if ci == NCH else xc[dc][:, ci * CH:(ci + 1) * CH]
               for dc in range(DC)]
        for dc in range(DC):
            nc.tensor.matmul(mu_ps[:], lhsT=sel5[ci], rhs=src[dc],
                             start=(ci == 0 and dc == 0),
                             stop=(ci == NCH and dc == DC - 1),
                             skip_group_check=True)
        for dc in range(DC):
            sq = psq.tile([128, CH], BF, tag="sq")
            if dc % 3 == 0:
                nc.vector.tensor_mul(sq[:], src[dc], src[dc])
            else:
                nc.scalar.activation(sq[:], src[dc], AF.Square)
            nc.tensor.matmul(msq_ps[:], lhsT=sel5[ci], rhs=sq[:],
                             start=(ci == 0 and dc == 0),
                             stop=(ci == NCH and dc == DC - 1),
                             skip_group_check=True)

    muR, SR, rs1R = _row_stats(nc, prow, eps5, mu_ps, msq_ps, 5)

    # broadcast + normalize per chunk
    for ci in range(NCH + 1):
        lhs = selbc[:, ci * 128:(ci + 1) * 128]
        mu_b = pbc.tile([128, CH], F32, tag="bc", name="mu_b")
        nc.tensor.matmul(mu_b[:], lhsT=lhs, rhs=muR[:], start=True, stop=True)
        S_b = pbc.tile([128, CH], F32, tag="bc", name="S_b")
        nc.tensor.matmul(S_b[:], lhsT=lhs, rhs=SR[:], start=True, stop=True)
        if ci == NCH:
            rs1_b = pbc.tile([128, CH], F32, tag="bc", name="rs1_b")
            nc.tensor.matmul(rs1_b[:], lhsT=lhs, rhs=rs1R[:], start=True, stop=True)
            for dc in range(DC):
                tmp = ptm.tile([128, CH], BF, tag="tmp")
                nc.vector.tensor_sub(tmp[:], xq[dc][:], mu_b[:])
                nc.vector.tensor_mul(nqT[dc][:], tmp[:], S_b[:])
                nc.vector.tensor_mul(y1T[dc][:], tmp[:], rs1_b[:])
        else:
            for dc in range(DC):
                tmp = ptm.tile([128, CH], BF, tag="tmp")
                nc.vector.tensor_sub(tmp[:], xc[dc][:, ci * CH:(ci + 1) * CH], mu_b[:])
                nc.vector.tensor_mul(nT[dc][:, ci * CH:(ci + 1) * CH], tmp[:], S_b[:])


def _phase_v(nc, tc, ctx, env):
    """V GEMM token-major with an interleaved ones column per head."""
    nT, VA, wq_sb = env["nT"], env["VA"], env["wq_sb"]
    ppsv = ctx.enter_context(tc.tile_pool(name="b_psv", bufs=2, space="PSUM"))

    for jt in range(NJT):
        nc.vector.memset(VA[jt][:].rearrange("p (h c) -> p h c", c=65)[:, :, 64:65], 1.0)
    for jt in range(NJT):
        jsz = JSZ[jt]
        ps_v = ppsv.tile([128, D], F32, tag="psv")
        for dc in range(DC):
            lhsT = nT[dc][:, jt * 128:jt * 128 + jsz]
            nc.tensor.matmul(ps_v[0:jsz, 0:512], lhsT=lhsT,
                             rhs=wq_sb[dc][:, 2 * D:2 * D + 512],
                             start=(dc == 0), stop=(dc == DC - 1),
                             skip_group_check=True)
            nc.tensor.matmul(ps_v[0:jsz, 512:D], lhsT=lhsT,
                             rhs=wq_sb[dc][:, 2 * D + 512:3 * D],
                             start=(dc == 0), stop=(dc == DC - 1),
                             skip_group_check=True)
        vav = VA[jt][:].rearrange("p (h c) -> p h c", c=65)
        eng = nc.vector.tensor_copy if jt % 2 == 0 else nc.scalar.copy
        eng(vav[0:jsz, :, 0:64],
            ps_v[0:jsz, :].rearrange("p (h c) -> p h c", c=64))


def _emit_kq(nc, env, pools, et):
    """Emit the K and Q GEMM units for e-tile `et` as a list of closures."""
    nT, nqT, KT, QT = env["nT"], env["nqT"], env["KT"], env["QT"]
    wq_sb, cbq_sb = env["wq_sb"], env["cbq_sb"]
    pmk, pmq = pools

    units = []
    for ci in range(NCH):
        def k_unit(ci=ci):
            ps = pmk.tile([128, CH], F32, tag="mmk")
            for dc in range(DC):
                nc.tensor.matmul(ps[:], lhsT=wq_sb[dc][:, D + et * 128:D + (et + 1) * 128],
                                 rhs=nT[dc][:, ci * CH:(ci + 1) * CH],
                                 start=(dc == 0), stop=(dc == DC - 1),
                                 skip_group_check=True)
            eng = nc.vector.tensor_copy if ci % 2 == 0 else nc.scalar.copy
            eng(KT[et][:, ci * CH:(ci + 1) * CH], ps[:])
        units.append(k_unit)

    def q_unit():
        ps = pmq.tile([128, NQ], F32, tag="mmk")
        for dc in range(DC):
            nc.tensor.matmul(ps[:], lhsT=wq_sb[dc][:, et * 128:(et + 1) * 128],
                             rhs=nqT[dc][:], start=(dc == 0), stop=(dc == DC - 1),
                             skip_group_check=True)
        nc.scalar.activation(QT[et][:], ps[:], AF.Identity, bias=cbq_sb[:, et:et + 1])
    units.append(q_unit)
    return units


def _phase_attn(nc, tc, ctx, env):
    """Per head-pair: scores, masked exp, PV with softmax sums via the ones
    column, normalization into ONT.  K/Q GEMMs for hp+1 are interleaved."""
    KT, QT, VA, ONT = env["KT"], env["QT"], env["VA"], env["ONT"]
    msk_sb = env["msk_sb"]
    ones64 = env["selbc1"][:, 0:64]

    ppt = ctx.enter_context(tc.tile_pool(name="c_pt", bufs=3))
    prr = ctx.enter_context(tc.tile_pool(name="c_rr", bufs=2))
    pss = ctx.enter_context(tc.tile_pool(name="c_ps_s", bufs=2, space="PSUM"))
    pso = ctx.enter_context(tc.tile_pool(name="c_ps_o", bufs=3, space="PSUM"))
    prb = ctx.enter_context(tc.tile_pool(name="c_ps_rb", bufs=1, space="PSUM"))
    pmk = ctx.enter_context(tc.tile_pool(name="c_mmk", bufs=2, space="PSUM"))
    pmq = pmk

    # prologue: K/Q for hp=0
    for u in _emit_kq(nc, env, (pmk, pmq), 0):
        u()

    for hp in range(6):
        fillers = _emit_kq(nc, env, (pmk, pmq), hp + 1) if hp < 5 else []
        o_ps = [pso.tile([65, NQ], F32, tag="pso", name=f"o_ps{i}") for i in range(2)]
        pts = [None] * NJT
        for jt in range(NJT):
            jsz = JSZ[jt]
            pt = ppt.tile([128, 2, NQ], BF, tag="pt")
            pts[jt] = pt
            for hi in range(2):
                part = 64 * hi
                s_ps = pss.tile([128, NQ], F32, tag="ps_s")
                nc.tensor.matmul(
                    s_ps[0:jsz, :],
                    lhsT=KT[hp][part:part + 64, jt * 128:jt * 128 + jsz],
                    rhs=QT[hp][part:part + 64, :], start=True, stop=True,
                    skip_group_check=True)
                nc.scalar.activation(
                    pt[0:jsz, hi, :], s_ps[0:jsz, :], AF.Exp,
                    bias=msk_sb[0:jsz, NJT + jt:NJT + jt + 1],
                    scale=msk_sb[0:jsz, jt:jt + 1])
                if jt >= JLO:
                    nc.vector.tensor_scalar_mul(
                        pt[0:jsz, hi, 0:NPATCH], pt[0:jsz, hi, 0:NPATCH],
                        msk_sb[0:jsz, 2 * NJT + jt:2 * NJT + jt + 1])
            if jt >= 1:
                pjt = jt - 1
                for hi in range(2):
                    nc.tensor.matmul(
                        o_ps[hi][:], lhsT=VA[pjt][0:JSZ[pjt], (2 * hp + hi) * 65:(2 * hp + hi + 1) * 65],
                        rhs=pts[pjt][0:JSZ[pjt], hi, :], start=(pjt == 0), stop=False,
                        skip_group_check=True)
                pts[pjt] = None
            if jt % 3 == 2 and fillers:
                fillers.pop(0)()
        for hi in range(2):
            nc.tensor.matmul(
                o_ps[hi][:], lhsT=VA[NJT - 1][0:JSZ[NJT - 1], (2 * hp + hi) * 65:(2 * hp + hi + 1) * 65],
                rhs=pts[NJT - 1][0:JSZ[NJT - 1], hi, :], start=False, stop=True,
                skip_group_check=True)
        for u in fillers:
            u()
        # normalize: rcp rows -> PE broadcast to PSUM -> SBUF copy -> ONT
        rcp_ps = prb.tile([128, NQ], F32, tag="rb")
        for hi in range(2):
            part = 64 * hi
            rcp = prr.tile([1, NQ], BF, tag="rr")
            nc.vector.reciprocal(rcp[:], o_ps[hi][64:65, :])
            nc.tensor.matmul(rcp_ps[part:part + 64, :], lhsT=ones64[:], rhs=rcp[:],
                             start=True, stop=True, skip_group_check=True)
        rcp_sb = prr.tile([128, NQ], BF, tag="rsb")
        nc.vector.tensor_copy(rcp_sb[:], rcp_ps[:])
        for hi in range(2):
            part = 64 * hi
            nc.vector.tensor_mul(ONT[hp][part:part + 64, :], o_ps[hi][0:64, :],
                                 rcp_sb[part:part + 64, :])


def _phase_outproj(nc, tc, ctx, env):
    wo_sb, ONT, y1T, x2T, x2b, bout_sb = (
        env["wo_sb"], env["ONT"], env["y1T"], env["x2T"], env["x2b"], env["bout_sb"])
    pmmD = ctx.enter_context(tc.tile_pool(name="d_mm", bufs=2, space="PSUM"))
    for dt in range(DC):
        ps = pmmD.tile([128, NQ], F32, tag="mmD")
        for et in range(DC):
            nc.tensor.matmul(ps[:], lhsT=wo_sb[et][:, dt * 128:(dt + 1) * 128],
                             rhs=ONT[et][:], start=(et == 0), stop=(et == DC - 1),
                             skip_group_check=True)
        nc.vector.scalar_tensor_tensor(
            x2T[dt][:], ps[:], bout_sb[:, dt:dt + 1], y1T[dt][:],
            op0=ALU.add, op1=ALU.add)
        nc.scalar.copy(x2b[dt][:], x2T[dt][:])


def _phase_ln2(nc, tc, ctx, env):
    """LN2 stats + normalize into n2T (bf16)."""
    x2b, n2T = env["x2b"], env["n2T"]
    ones1, selbc1, eps5 = env["ones1"], env["selbc1"], env["eps5"]

    psq2 = ctx.enter_context(tc.tile_pool(name="e_sq", bufs=4))
    prow2 = ctx.enter_context(tc.tile_pool(name="e_rows", bufs=8))
    pst2 = ctx.enter_context(tc.tile_pool(name="e_st", bufs=2, space="PSUM"))
    pbc2 = ctx.enter_context(tc.tile_pool(name="e_bc", bufs=2, space="PSUM"))

    mu_ps = pst2.tile([1, NQ], F32, tag="st2", name="mu2_ps")
    msq_ps = pst2.tile([1, NQ], F32, tag="st2", name="msq2_ps")
    for dc in range(DC):
        nc.tensor.matmul(mu_ps[:], lhsT=ones1[:], rhs=x2b[dc][:],
                         start=(dc == 0), stop=(dc == DC - 1), skip_group_check=True)
    for dc in range(DC):
        sq = psq2.tile([128, NQ], BF, tag="sq2")
        if dc % 2 == 0:
            nc.vector.tensor_mul(sq[:], x2b[dc][:], x2b[dc][:])
        else:
            nc.scalar.activation(sq[:], x2b[dc][:], AF.Square)
        nc.tensor.matmul(msq_ps[:], lhsT=ones1[:], rhs=sq[:],
                         start=(dc == 0), stop=(dc == DC - 1), skip_group_check=True)

    invD = 1.0 / D
    muR = prow2.tile([1, NQ], BF, tag="row2", name="mu2R")
    nc.vector.tensor_scalar_mul(muR[:], mu_ps[:], invD)
    musq = prow2.tile([1, NQ], F32, tag="row2", name="musq2")
    nc.vector.tensor_mul(musq[:], muR[:], muR[:])
    var = prow2.tile([1, NQ], F32, tag="row2", name="var2")
    nc.vector.scalar_tensor_tensor(var[:], msq_ps[:], invD, musq[:],
                                   op0=ALU.mult, op1=ALU.subtract)
    nc.scalar.activation(var[:], var[:], AF.Sqrt, bias=eps5[0:1, 0:1])
    SR = prow2.tile([1, NQ], BF, tag="row2", name="S2R")
    nc.vector.reciprocal(SR[:], var[:])

    mu_b = pbc2.tile([128, NQ], F32, tag="bc2")
    nc.tensor.matmul(mu_b[:], lhsT=selbc1[:], rhs=muR[:], start=True, stop=True)
    S_b = pbc2.tile([128, NQ], F32, tag="bc2")
    nc.tensor.matmul(S_b[:], lhsT=selbc1[:], rhs=SR[:], start=True, stop=True)
    for dc in range(DC):
        tmp = psq2.tile([128, NQ], BF, tag="sq2")
        nc.vector.tensor_sub(tmp[:], x2b[dc][:], mu_b[:])
        nc.vector.tensor_mul(n2T[dc][:], tmp[:], S_b[:])


def _phase_ffn(nc, tc, ctx, env):
    """Interleaved FFN1(silu)/FFN2 with streamed w2 tiles."""
    w1_sb, w2T, cb1_sb = env["w1_sb"], env["w2T"], env["cb1_sb"]
    n2T = env["n2T"]

    pw2 = ctx.enter_context(tc.tile_pool(name="f_w2", bufs=4))
    pffs = ctx.enter_context(tc.tile_pool(name="f_ffs", bufs=3))
    pmmE = ctx.enter_context(tc.tile_pool(name="f_mm", bufs=2, space="PSUM"))
    ps_acc = env["ps_acc"]
    for ft in range(FT):
        w2t = pw2.tile([128, D], BF, tag="w2")
        nc.sync.dma_start(w2t[:], w2T[ft * 128:(ft + 1) * 128, :])
        ps1 = pmmE.tile([128, NQ], F32, tag="mm1")
        for dc in range(DC):
            nc.tensor.matmul(ps1[:], lhsT=w1_sb[dc][:, ft * 128:(ft + 1) * 128],
                             rhs=n2T[dc][:], start=(dc == 0), stop=(dc == DC - 1),
                             skip_group_check=True)
        # silu(u) = u * sigmoid(u) with u = ps1 + cb1
        sig = pffs.tile([128, NQ], F32, tag="sig")
        nc.scalar.activation(sig[:], ps1[:], AF.Sigmoid, bias=cb1_sb[:, ft:ft + 1])
        ffs = pffs.tile([128, NQ], BF, tag="ffs")
        nc.vector.scalar_tensor_tensor(ffs[:], ps1[:], cb1_sb[:, ft:ft + 1], sig[:],
                                       op0=ALU.add, op1=ALU.mult)
        for dt in range(DC):
            nc.tensor.matmul(ps_acc[dt][:], lhsT=w2t[:, dt * 128:(dt + 1) * 128],
                             rhs=ffs[:], start=(ft == 0), stop=(ft == FT - 1),
                             skip_group_check=True)


def _phase_store(nc, tc, ctx, env):
    """Residual add, transpose feature-major result to token-major, store."""
    ps_acc, x2T, b2_sb = env["ps_acc"], env["x2T"], env["b2_sb"]
    identb, out = env["identb"], env["out"]
    pout = ctx.enter_context(tc.tile_pool(name="h_out", bufs=DC))
    posb = ctx.enter_context(tc.tile_pool(name="h_osb", bufs=2))
    ptr = ctx.enter_context(tc.tile_pool(name="h_tr", bufs=2, space="PSUM"))

    outT = []
    for dt in range(DC):
        t = pout.tile([128, NQ], BF, tag="outT", name=f"outT{dt}")
        nc.vector.scalar_tensor_tensor(
            t[:], ps_acc[dt][:], b2_sb[:, dt:dt + 1], x2T[dt][:],
            op0=ALU.add, op1=ALU.add)
        outT.append(t)
    QSZ = [128, 128, 128, 8]
    for qt in range(4):
        qsz = QSZ[qt]
        osb = posb.tile([128, D], F32, tag="osb")
        for dt in range(DC):
            tp = ptr.tile([128, 128], BF, tag="ptr")
            nc.tensor.transpose(tp[0:qsz, :],
                                outT[dt][:, qt * 128:qt * 128 + qsz], identb[:])
            nc.scalar.copy(osb[0:qsz, dt * 128:(dt + 1) * 128], tp[0:qsz, :])
        nc.sync.dma_start(out[qt * 128:qt * 128 + qsz, :], osb[0:qsz, :])


def build_program():
    nc = bacc.Bacc("TRN2")
    env = {}
    env["xT"] = nc.declare_dram_parameter("xT", [D, L], BF, isOutput=False)
    env["xqT"] = nc.declare_dram_parameter("xqT", [D, NQ], BF, isOutput=False)
    wqkvT = nc.declare_dram_parameter("wqkvT", [D, 3 * D], BF, isOutput=False)
    cbq = nc.declare_dram_parameter("cbq", [128, DC], F32, isOutput=False)
    woutT = nc.declare_dram_parameter("woutT", [D, D], BF, isOutput=False)
    bout = nc.declare_dram_parameter("bout", [128, DC], F32, isOutput=False)
    w1T = nc.declare_dram_parameter("w1T", [D, DFF], BF, isOutput=False)
    cb1 = nc.declare_dram_parameter("cb1", [128, FT], F32, isOutput=False)
    env["w2T"] = nc.declare_dram_parameter("w2T", [DFF, D], BF, isOutput=False)
    b2 = nc.declare_dram_parameter("b2", [128, DC], F32, isOutput=False)
    msk = nc.declare_dram_parameter("msk", [128, 3 * NJT], F32, isOutput=False)
    selc = nc.declare_dram_parameter("selc", [128, 5 * (NCH + 1)], BF, isOutput=False)
    selbcp = nc.declare_dram_parameter("selbcp", [5, 640], BF, isOutput=False)
    env["out"] = nc.declare_dram_parameter("out", [NQ, D], F32, isOutput=True)

    with tile.TileContext(nc) as tc, ExitStack() as top:
        top.enter_context(nc.allow_low_precision(
            reason="bf16 intermediates; rel-err gate is 2e-2"))
        pc = top.enter_context(tc.tile_pool(name="const", bufs=1))
        pw = top.enter_context(tc.tile_pool(name="weights", bufs=DC))
        pact = top.enter_context(tc.tile_pool(name="acts", bufs=DC))
        pmid = top.enter_context(ExitStack())

        pnt = pmid.enter_context(tc.tile_pool(name="ntp", bufs=DC))
        pnq = pmid.enter_context(tc.tile_pool(name="nqp", bufs=DC))
        env["nT"] = [pnt.tile([128, L], BF, tag="nt", name=f"nT{i}") for i in range(DC)]
        env["nqT"] = [pnq.tile([128, NQ], BF, tag="nq", name=f"nqT{i}") for i in range(DC)]
        # input DMAs first: LN1 needs them before any weight; per-chunk
        # pieces so stats matmuls can chase the stream
        for ci in range(NCH):
            for dc in range(DC):
                nc.sync.dma_start(
                    env["nT"][dc][:, ci * CH:(ci + 1) * CH],
                    env["xT"][dc * 128:(dc + 1) * 128, ci * CH:(ci + 1) * CH])
        for dc in range(DC):
            nc.sync.dma_start(env["nqT"][dc][:], env["xqT"][dc * 128:(dc + 1) * 128, :])

        # -- weight prefetch (big per-dc tiles) --
        env["wq_sb"] = []
        for dc in range(DC):
            t = pw.tile([128, 3 * D], BF, tag="wq", name=f"wq{dc}")
            nc.sync.dma_start(t[:], wqkvT[dc * 128:(dc + 1) * 128, :])
            env["wq_sb"].append(t)
        env["wo_sb"] = []
        for dc in range(DC):
            t = pw.tile([128, D], BF, tag="wo", name=f"wo{dc}")
            nc.sync.dma_start(t[:], woutT[dc * 128:(dc + 1) * 128, :])
            env["wo_sb"].append(t)
        env["w1_sb"] = []
        for dc in range(DC):
            t = pw.tile([128, DFF], BF, tag="w1", name=f"w1{dc}")
            nc.sync.dma_start(t[:], w1T[dc * 128:(dc + 1) * 128, :])
            env["w1_sb"].append(t)

        # -- constants --
        for name, prm, w in (("cbq_sb", cbq, DC), ("bout_sb", bout, DC),
                             ("b2_sb", b2, DC), ("cb1_sb", cb1, FT),
                             ("msk_sb", msk, 3 * NJT)):
            t = pc.tile([128, w], F32, tag=name, name=name)
            nc.sync.dma_start(t[:], prm[:])
            env[name] = t
        selc_sb = pc.tile([128, 5 * (NCH + 1)], BF, tag="selc", name="selc_sb")
        nc.sync.dma_start(selc_sb[:], selc[:])
        env["sel5"] = [selc_sb[:, 5 * ci:5 * (ci + 1)] for ci in range(NCH + 1)]
        selbc = pc.tile([5, 640], BF, tag="selbc", name="selbc")
        nc.sync.dma_start(selbc[:], selbcp[:])
        env["selbc"] = selbc
        selbc1 = pc.tile([1, 128], BF, tag="selbc1", name="selbc1")
        nc.vector.memset(selbc1[:], 1.0)
        env["selbc1"] = selbc1
        ones1 = pc.tile([128, 1], BF, tag="ones1", name="ones1")
        nc.vector.memset(ones1[:], 1.0)
        env["ones1"] = ones1
        eps5 = pc.tile([5, 1], F32, tag="eps5", name="eps5")
        nc.vector.memset(eps5[:], EPS)
        env["eps5"] = eps5
        ident = pc.tile([128, 128], F32, tag="ident", name="ident")
        make_identity(nc, ident[:])
        identb = pc.tile([128, 128], BF, tag="identb", name="identb")
        nc.vector.tensor_copy(identb[:], ident[:])
        env["identb"] = identb

        # -- persistent activations --
        env["y1T"] = [pact.tile([128, NQ], BF, tag="y1", name=f"y1T{i}") for i in range(DC)]
        env["x2T"] = [pact.tile([128, NQ], F32, tag="x2", name=f"x2T{i}") for i in range(DC)]
        env["x2b"] = [pact.tile([128, NQ], BF, tag="x2b", name=f"x2b{i}") for i in range(DC)]
        env["n2T"] = [pact.tile([128, NQ], BF, tag="n2", name=f"n2T{i}") for i in range(DC)]

        with ExitStack() as mid:
            pkt = mid.enter_context(tc.tile_pool(name="ktp", bufs=DC))
            pqt = mid.enter_context(tc.tile_pool(name="qtp", bufs=DC))
            pva = mid.enter_context(tc.tile_pool(name="vap", bufs=NJT))
            pont = mid.enter_context(tc.tile_pool(name="ontp", bufs=DC))
            env["KT"] = [pkt.tile([128, L], BF, tag="kt", name=f"KT{i}") for i in range(DC)]
            env["QT"] = [pqt.tile([128, NQ], BF, tag="qt", name=f"QT{i}") for i in range(DC)]
            env["VA"] = [pva.tile([128, H * 65], BF, tag="va", name=f"VA{i}") for i in range(NJT)]
            env["ONT"] = [pont.tile([128, NQ], BF, tag="ont", name=f"ONT{i}") for i in range(DC)]

            with ExitStack() as ctx:
                _phase_ln1(nc, tc, ctx, env)
            with ExitStack() as ctx:
                _phase_v(nc, tc, ctx, env)
            with ExitStack() as ctx:
                _phase_attn(nc, tc, ctx, env)
            with ExitStack() as ctx:
                _phase_outproj(nc, tc, ctx, env)
        pmid.close()

        with ExitStack() as ctx:
            _phase_ln2(nc, tc, ctx, env)
        with ExitStack() as ctx:
            pacc = ctx.enter_context(tc.tile_pool(name="f_acc", bufs=DC, space="PSUM"))
            env["ps_acc"] = [pacc.tile([128, NQ], F32, tag="acc", name=f"ps_acc{i}")
                             for i in range(DC)]
            with ExitStack() as ctx2:
                _phase_ffn(nc, tc, ctx2, env)
            with ExitStack() as ctx2:
                _phase_store(nc, tc, ctx2, env)

    nc.finalize()
    return nc


_NC = None


def _get_nc():
    global _NC
    if _NC is None:
        _NC = build_program()
    return _NC


def _host_prepare(inputs):
    """Fold constants and lay out per-core input maps."""
    import ml_dtypes
    bf16 = ml_dtypes.bfloat16
    f32 = np.float32
    x = np.asarray(inputs["x"], f32)
    memory = np.asarray(inputs["memory"], f32)
    w_qkv = np.asarray(inputs["w_qkv"], f32)
    w_out = np.asarray(inputs["w_out"], f32)
    b_out = np.asarray(inputs["b_out"], f32)
    g_att = np.asarray(inputs["ln_att_g"], f32)
    b_att = np.asarray(inputs["ln_att_b"], f32)
    g2 = np.asarray(inputs["ln2_g"], f32)
    bb2 = np.asarray(inputs["ln2_b"], f32)
    w1 = np.asarray(inputs["w1"], f32)
    b1 = np.asarray(inputs["b1"], f32)
    w2 = np.asarray(inputs["w2"], f32)
    b2v = np.asarray(inputs["b2"], f32)

    qscale = f32(DH ** -0.5)
    w_qkv_eff = w_qkv * g_att[None, :]
    w_qkv_eff[:D] *= qscale
    cb_qkv = w_qkv @ b_att
    cb_q = (cb_qkv[:D] * qscale).astype(f32)
    cb_v = cb_qkv[2 * D:].astype(f32)
    b_out_eff = (b_out + w_out @ cb_v).astype(f32)
    w1_eff = w1 * g2[None, :]
    cb1_eff = (w1 @ bb2 + b1).astype(f32)

    def cols(v):
        # [N] vector -> [128, N//128] per-partition bias layout
        return np.ascontiguousarray(v.reshape(-1, 128).T)

    selc = np.zeros((128, 5 * (NCH + 1)), np.float32)
    for ci in range(NCH + 1):
        selc[:, 5 * ci + ci] = 1.0
    selbcp = np.zeros((5, 640), np.float32)
    for k in range(5):
        selbcp[k, k * 128:(k + 1) * 128] = 1.0
    shared = {
        "selc": selc.astype(bf16),
        "selbcp": selbcp.astype(bf16),
        "wqkvT": np.ascontiguousarray(w_qkv_eff.T).astype(bf16),
        "cbq": cols(cb_q),
        "woutT": np.ascontiguousarray(w_out.T).astype(bf16),
        "bout": cols(b_out_eff),
        "w1T": np.ascontiguousarray(w1_eff.T).astype(bf16),
        "cb1": cols(cb1_eff),
        "w2T": np.ascontiguousarray(w2.T).astype(bf16),
        "b2": cols(b2v),
    }

    in_maps = []
    for c in range(NCORES):
        b, hf = divmod(c, 2)
        x_aug = np.concatenate([memory[b, :T], x[b]], axis=0)      # [L, D]
        q0 = T + hf * NQ
        LcA = (5 + 2 * hf) * NPATCH
        LcB = (6 + 2 * hf) * NPATCH
        j = np.arange(NJT * 128)
        sa = (j < LcB).astype(f32)
        ba = np.where(sa > 0, 0.0, -30.0).astype(f32)
        ma = (j < LcA).astype(f32)
        mskv = np.concatenate(
            [v.reshape(NJT, 128).T for v in (sa, ba, ma)], axis=1)
        in_maps.append({
            "xT": np.ascontiguousarray(x_aug.T).astype(bf16),
            "xqT": np.ascontiguousarray(x_aug[q0:q0 + NQ].T).astype(bf16),
            "msk": np.ascontiguousarray(mskv),
            **shared,
        })
    return in_maps


def _assemble(results):
    out = np.zeros((B, T, D), np.float32)
    for c in range(NCORES):
        b, hf = divmod(c, 2)
        out[b, hf * NQ:(hf + 1) * NQ, :] = results[c]["out"]
    return out


def kernel(**inputs):
    nc = _get_nc()
    in_maps = _host_prepare(inputs)
    res = run_bass_kernel_spmd(nc, in_maps, list(range(NCORES)))
    return _assemble(res.results)


def _ensure_ntff_hook():
    """Provide antenv.axon_hooks (absent in this image) so trace=True can
    drive NTFF capture through libaxon_pjrt.so, mirroring trn_boot.py."""
    import contextlib
    import ctypes
    import types

    try:
        from antenv.axon_hooks import get_axon_ntff_profile_hook  # noqa: F401
        return
    except ImportError:
        pass
    import antenv

    so_path = "/opt/axon/libaxon_pjrt.so"
    lib = ctypes.CDLL(so_path)
    if not hasattr(lib, "axon_start_nrt_profile"):
        raise RuntimeError("libaxon_pjrt.so lacks NTFF profile symbols")
    lib.axon_start_nrt_profile.argtypes = [ctypes.POINTER(ctypes.c_int64),
                                           ctypes.c_size_t]
    lib.axon_start_nrt_profile.restype = ctypes.c_int64
    lib.axon_stop_nrt_profile.argtypes = [ctypes.c_char_p]
    lib.axon_stop_nrt_profile.restype = ctypes.c_int64

    @contextlib.contextmanager
    def _hook(output_dir, device_ids):
        import jax
        jax.devices()
        if device_ids:
            ids = (ctypes.c_int64 * len(device_ids))(*device_ids)
            rc = lib.axon_start_nrt_profile(ids, len(device_ids))
        else:
            rc = lib.axon_start_nrt_profile(None, 0)
        if rc != 0:
            raise RuntimeError(f"axon_start_nrt_profile rc={rc}")
        try:
            yield
        finally:
            n = lib.axon_stop_nrt_profile(str(output_dir).encode())
            print(f"ntff profile: {n} file(s) written to {output_dir}",
                  file=sys.stderr)

    box = {"h": _hook}
    mod = types.ModuleType("antenv.axon_hooks")
    mod.set_axon_ntff_profile_hook = lambda h: box.__setitem__("h", h)
    mod.get_axon_ntff_profile_hook = lambda: box["h"]
    sys.modules["antenv.axon_hooks"] = mod
    antenv.axon_hooks = mod


def kernel_traced(**inputs):
    """Like kernel() but with NTFF profiling; returns (out, exec_time_ns)."""
    import tempfile

    from concourse import bass_utils as _bu
    _ensure_ntff_hook()
    _bu.upload_artifacts = lambda tmpdir: f"local:{tmpdir}"  # no bucket creds here
    nc = _get_nc()
    in_maps = _host_prepare(inputs)
    tmpdir = tempfile.mkdtemp(prefix="ntff_")
    res = run_bass_kernel_spmd(nc, in_maps, list(range(NCORES)), trace=True,
                               tmpdir=tmpdir)
    return _assemble(res.results), res.exec_time_ns


# revision 14
# speedup vs baseline: 2.1060x; 1.0434x over previous
"""Trainium2 Bass kernel: LookupTransformerBlock (block-causal sparse attention).

Reference semantics (B=4, T=784, D=768, H=12, Dh=64, d_ff=3072):
  x_aug = LN1(concat(memory[:, :T], x))              # [B, 2T, D], ln1 g=1/b=0
  h     = LN_att(x_aug)
  qkv   = h @ w_qkv.T ; block-causal attention over frames of 196
  x2    = x_aug + attn_out
  out   = (x2 + FFN(LN2(x2)))[:, T:, :]

Sharding: 8 cores = (batch b in 0..3) x (query-half hf in 0..1); each core
computes its 392 output rows with K/V over all 1568 positions (data-parallel,
no collectives).  One SPMD program; per-core differences (query slice, mask
extents) live in input data only.

v2 design (vs the fp32r baseline):
  - all matmul operands bf16 (1-pass PE, half DMA), fp32 PSUM accumulation
  - no DRAM-bounce row broadcasts: rows are broadcast across partitions with
    tiny selector matmuls on the PE (ones-column outer products)
  - LN1 stats for all 4 L-chunks + the query slice accumulate into one
    [5, 392] PSUM pair via per-chunk selector columns; row math runs once
  - weights DMA'd in 6 big per-dc tiles each, prefetched at kernel start
  - frame-A mask boundary applied as a per-partition DVE multiply on the
    exp tile instead of a second Exp on the scalar engine
  - K/Q GEMMs for head-pair hp+1 are software-pipelined into the attention
    j-loop of head-pair hp so the PE never idles long enough to re-throttle
"""

import os
import sys
from contextlib import ExitStack

import numpy as np

for _p in ("/opt/trn_rl_repo", os.path.expanduser("~/.axon_site/_ro/trn_rl_repo")):
    if os.path.isdir(_p) and _p not in sys.path:
        sys.path.append(_p)

import concourse.bass as bass
import concourse.bacc as bacc
import concourse.mybir as mybir
import concourse.tile as tile
from concourse.bass_utils import run_bass_kernel_spmd
from concourse.masks import make_identity

F32 = mybir.dt.float32
BF = mybir.dt.bfloat16
AF = mybir.ActivationFunctionType
ALU = mybir.AluOpType

B = 4
T = 784
D = 768
L = 2 * T            # 1568
NQ = 392             # query rows per core
H = 12
DH = 64
DFF = 3072
NPATCH = 196
DC = D // 128        # 6
FT = DFF // 128      # 24
NJT = 13             # j-tiles over L (12 x 128 + 32)
JSZ = [128] * 12 + [32]
CH = 392             # LN1 l-chunk width (L = 4*CH)
NCH = 4
EPS = 1e-5
NCORES = 8
JLO = 7              # first j-tile that can contain a frame-A mask boundary


def _row_stats(nc, prow, eps5, mu_ps, msq_ps, nrow):
    """Row math on [nrow, CH] stats: returns (muR, SR, rs1R) bf16 rows.

    mu = sum/D;  var = sumsq/D - mu^2;  rs1 = 1/sqrt(var+eps);
    rs2 = 1/sqrt(var*rs1^2+eps);  S = rs1*rs2  (fused LN1+LN_att scale).
    """
    invD = 1.0 / D
    muR = prow.tile([nrow, CH], BF, tag="row", name="muR")
    nc.vector.tensor_scalar_mul(muR[:], mu_ps[:], invD)
    musq = prow.tile([nrow, CH], F32, tag="row", name="musq")
    nc.vector.tensor_mul(musq[:], muR[:], muR[:])
    var = prow.tile([nrow, CH], F32, tag="row", name="var")
    nc.vector.scalar_tensor_tensor(var[:], msq_ps[:], invD, musq[:],
                                   op0=ALU.mult, op1=ALU.subtract)
    sd = prow.tile([nrow, CH], F32, tag="row", name="sd")
    nc.scalar.activation(sd[:], var[:], AF.Sqrt, bias=eps5[0:nrow, 0:1])
    rs1R = prow.tile([nrow, CH], BF, tag="row", name="rs1R")
    nc.vector.reciprocal(rs1R[:], sd[:])
    v2 = prow.tile([nrow, CH], F32, tag="row", name="v2")
    nc.vector.tensor_mul(v2[:], rs1R[:], rs1R[:])
    nc.vector.tensor_mul(v2[:], var[:], v2[:])
    nc.scalar.activation(v2[:], v2[:], AF.Sqrt, bias=eps5[0:nrow, 0:1])
    rs2 = prow.tile([nrow, CH], F32, tag="row", name="rs2")
    nc.vector.reciprocal(rs2[:], v2[:])
    SR = prow.tile([nrow, CH], BF, tag="row", name="SR")
    nc.vector.tensor_mul(SR[:], rs1R[:], rs2[:])
    return muR, SR, rs1R


def _phase_ln1(nc, tc, ctx, env):
    """LN1+LN_att fused normalization of x_aug (4 chunks) and the q slice.

    Raw x is DMA'd straight into the nT/nqT tiles and normalized in place.
    """
    sel5, selbc, eps5 = env["sel5"], env["selbc"], env["eps5"]
    nT, nqT, y1T = env["nT"], env["nqT"], env["y1T"]

    psq = ctx.enter_context(tc.tile_pool(name="a_sq", bufs=6))
    ptm = ctx.enter_context(tc.tile_pool(name="a_tmp", bufs=4))
    prow = ctx.enter_context(tc.tile_pool(name="a_rows", bufs=8))
    pst = ctx.enter_context(tc.tile_pool(name="a_st", bufs=2, space="PSUM"))
    pbc = ctx.enter_context(tc.tile_pool(name="a_bc", bufs=4, space="PSUM"))

    xc, xq = nT, nqT

    mu_ps = pst.tile([5, CH], F32, tag="st", name="mu_ps")
    msq_ps = pst.tile([5, CH], F32, tag="st", name="msq_ps")
    for ci in range(NCH + 1):
        src = [xq[dc][:] if ci == NCH else xc[dc][:, ci * CH:(ci + 1) * CH]
               for dc in range(DC)]
        for dc in range(DC):
            nc.tensor.matmul(mu_ps[:], lhsT=sel5[ci], rhs=src[dc],
                             start=(ci == 0 and dc == 0),
                             stop=(ci == NCH and dc == DC - 1),
                             skip_group_check=True)
        for dc in range(DC):
            sq = psq.tile([128, CH], BF, tag="sq")
            if dc % 3 == 0:
                nc.vector.tensor_mul(sq[:], src[dc], src[dc])
            else:
                nc.scalar.activation(sq[:], src[dc], AF.Square)
            nc.tensor.matmul(msq_ps[:], lhsT=sel5[ci], rhs=sq[:],
                             start=(ci == 0 and dc == 0),
                             stop=(ci == NCH and dc == DC - 1),
                             skip_group_check=True)

    muR, SR, rs1R = _row_stats(nc, prow, eps5, mu_ps, msq_ps, 5)

    # broadcast + normalize per chunk
    for ci in range(NCH + 1):
        lhs = selbc[:, ci * 128:(ci + 1) * 128]
        mu_b = pbc.tile([128, CH], F32, tag="bc", name="mu_b")
        nc.tensor.matmul(mu_b[:], lhsT=lhs, rhs=muR[:], start=True, stop=True)
        S_b = pbc.tile([128, CH], F32, tag="bc", name="S_b")
        nc.tensor.matmul(S_b[:], lhsT=lhs, rhs=SR[:], start=True, stop=True)
        if ci == NCH:
            rs1_b = pbc.tile([128, CH], F32, tag="bc", name="rs1_b")
            nc.tensor.matmul(rs1_b[:], lhsT=lhs, rhs=rs1R[:], start=True, stop=True)
            for dc in range(DC):
                tmp = ptm.tile([128, CH], BF, tag="tmp")
                nc.vector.tensor_sub(tmp[:], xq[dc][:], mu_b[:])
                nc.vector.tensor_mul(nqT[dc][:], tmp[:], S_b[:])
                nc.vector.tensor_mul(y1T[dc][:], tmp[:], rs1_b[:])
        else:
            for dc in range(DC):
                tmp = ptm.tile([128, CH], BF, tag="tmp")
                nc.vector.tensor_sub(tmp[:], xc[dc][:, ci * CH:(ci + 1) * CH], mu_b[:])
                nc.vector.tensor_mul(nT[dc][:, ci * CH:(ci + 1) * CH], tmp[:], S_b[:])


def _phase_v(nc, tc, ctx, env):
    """V GEMM token-major with an interleaved ones column per head."""
    nT, VA, wq_sb = env["nT"], env["VA"], env["wq_sb"]
    ppsv = ctx.enter_context(tc.tile_pool(name="b_psv", bufs=2, space="PSUM"))

    for jt in range(NJT):
        nc.vector.memset(VA[jt][:].rearrange("p (h c) -> p h c", c=65)[:, :, 64:65], 1.0)
    for jt in range(NJT):
        jsz = JSZ[jt]
        ps_v = ppsv.tile([128, D], F32, tag="psv")
        for dc in range(DC):
            lhsT = nT[dc][:, jt * 128:jt * 128 + jsz]
            nc.tensor.matmul(ps_v[0:jsz, 0:512], lhsT=lhsT,
                             rhs=wq_sb[dc][:, 2 * D:2 * D + 512],
                             start=(dc == 0), stop=(dc == DC - 1),
                             skip_group_check=True)
            nc.tensor.matmul(ps_v[0:jsz, 512:D], lhsT=lhsT,
                             rhs=wq_sb[dc][:, 2 * D + 512:3 * D],
                             start=(dc == 0), stop=(dc == DC - 1),
                             skip_group_check=True)
        vav = VA[jt][:].rearrange("p (h c) -> p h c", c=65)
        eng = nc.vector.tensor_copy if jt % 2 == 0 else nc.scalar.copy
        eng(vav[0:jsz, :, 0:64],
            ps_v[0:jsz, :].rearrange("p (h c) -> p h c", c=64))


def _emit_kq(nc, env, pools, et):
    """Emit the K and Q GEMM units for e-tile `et` as a list of closures."""
    nT, nqT, KT, QT = env["nT"], env["nqT"], env["KT"], env["QT"]
    wq_sb, cbq_sb = env["wq_sb"], env["cbq_sb"]
    pmk, pmq = pools

    units = []
    for ci in range(NCH):
        def k_unit(ci=ci):
            ps = pmk.tile([128, CH], F32, tag="mmk")
            for dc in range(DC):
                nc.tensor.matmul(ps[:], lhsT=wq_sb[dc][:, D + et * 128:D + (et + 1) * 128],
                                 rhs=nT[dc][:, ci * CH:(ci + 1) * CH],
                                 start=(dc == 0), stop=(dc == DC - 1),
                                 skip_group_check=True)
            eng = nc.vector.tensor_copy if ci % 2 == 0 else nc.scalar.copy
            eng(KT[et][:, ci * CH:(ci + 1) * CH], ps[:])
        units.append(k_unit)

    def q_unit():
        ps = pmq.tile([128, NQ], F32, tag="mmk")
        for dc in range(DC):
            nc.tensor.matmul(ps[:], lhsT=wq_sb[dc][:, et * 128:(et + 1) * 128],
                             rhs=nqT[dc][:], start=(dc == 0), stop=(dc == DC - 1),
                             skip_group_check=True)
        nc.scalar.activation(QT[et][:], ps[:], AF.Identity, bias=cbq_sb[:, et:et + 1])
    units.append(q_unit)
    return units


def _phase_attn(nc, tc, ctx, env):
    """Per head-pair: scores, masked exp, PV with softmax sums via the ones
    column, normalization into ONT.  K/Q GEMMs for hp+1 are interleaved."""
    KT, QT, VA, ONT = env["KT"], env["QT"], env["VA"], env["ONT"]
    msk_sb = env["msk_sb"]
    ones64 = env["selbc1"][:, 0:64]

    ppt = ctx.enter_context(tc.tile_pool(name="c_pt", bufs=3))
    prr = ctx.enter_context(tc.tile_pool(name="c_rr", bufs=2))
    pss = ctx.enter_context(tc.tile_pool(name="c_ps_s", bufs=2, space="PSUM"))
    pso = ctx.enter_context(tc.tile_pool(name="c_ps_o", bufs=3, space="PSUM"))
    prb = ctx.enter_context(tc.tile_pool(name="c_ps_rb", bufs=1, space="PSUM"))
    pmk = ctx.enter_context(tc.tile_pool(name="c_mmk", bufs=2, space="PSUM"))
    pmq = pmk

    # prologue: K/Q for hp=0
    for u in _emit_kq(nc, env, (pmk, pmq), 0):
        u()

    for hp in range(6):
        fillers = _emit_kq(nc, env, (pmk, pmq), hp + 1) if hp < 5 else []
        o_ps = [pso.tile([65, NQ], F32, tag="pso", name=f"o_ps{i}") for i in range(2)]
        pts = [None] * NJT
        for jt in range(NJT):
            jsz = JSZ[jt]
            pt = ppt.tile([128, 2, NQ], BF, tag="pt")
            pts[jt] = pt
            for hi in range(2):
                part = 64 * hi
                s_ps = pss.tile([128, NQ], F32, tag="ps_s")
                nc.tensor.matmul(
                    s_ps[0:jsz, :],
                    lhsT=KT[hp][part:part + 64, jt * 128:jt * 128 + jsz],
                    rhs=QT[hp][part:part + 64, :], start=True, stop=True,
                    skip_group_check=True)
                nc.scalar.activation(
                    pt[0:jsz, hi, :], s_ps[0:jsz, :], AF.Exp,
                    bias=msk_sb[0:jsz, NJT + jt:NJT + jt + 1],
                    scale=msk_sb[0:jsz, jt:jt + 1])
                if jt >= JLO:
                    nc.vector.tensor_scalar_mul(
                        pt[0:jsz, hi, 0:NPATCH], pt[0:jsz, hi, 0:NPATCH],
                        msk_sb[0:jsz, 2 * NJT + jt:2 * NJT + jt + 1])
            if jt >= 1:
                pjt = jt - 1
                for hi in range(2):
                    nc.tensor.matmul(
                        o_ps[hi][:], lhsT=VA[pjt][0:JSZ[pjt], (2 * hp + hi) * 65:(2 * hp + hi + 1) * 65],
                        rhs=pts[pjt][0:JSZ[pjt], hi, :], start=(pjt == 0), stop=False,
                        skip_group_check=True)
                pts[pjt] = None
            if jt % 3 == 2 and fillers:
                fillers.pop(0)()
        for hi in range(2):
            nc.tensor.matmul(
                o_ps[hi][:], lhsT=VA[NJT - 1][0:JSZ[NJT - 1], (2 * hp + hi) * 65:(2 * hp + hi + 1) * 65],
                rhs=pts[NJT - 1][0:JSZ[NJT - 1], hi, :], start=False, stop=True,
                skip_group_check=True)
        for u in fillers:
            u()
        # normalize: rcp rows -> PE broadcast to PSUM -> SBUF copy -> ONT
        rcp_ps = prb.tile([128, NQ], F32, tag="rb")
        for hi in range(2):
            part = 64 * hi
            rcp = prr.tile([1, NQ], BF, tag="rr")
            nc.vector.reciprocal(rcp[:], o_ps[hi][64:65, :])
            nc.tensor.matmul(rcp_ps[part:part + 64, :], lhsT=ones64[:], rhs=rcp[:],
                             start=True, stop=True, skip_group_check=True)
        rcp_sb = prr.tile([128, NQ], BF, tag="rsb")
        nc.vector.tensor_copy(rcp_sb[:], rcp_ps[:])
        for hi in range(2):
            part = 64 * hi
            nc.vector.tensor_mul(ONT[hp][part:part + 64, :], o_ps[hi][0:64, :],
                                 rcp_sb[part:part + 64, :])


def _phase_outproj(nc, tc, ctx, env):
    wo_sb, ONT, y1T, x2T, x2b, bout_sb = (
        env["wo_sb"], env["ONT"], env["y1T"], env["x2T"], env["x2b"], env["bout_sb"])
    pmmD = ctx.enter_context(tc.tile_pool(name="d_mm", bufs=2, space="PSUM"))
    for dt in range(DC):
        ps = pmmD.tile([128, NQ], F32, tag="mmD")
        for et in range(DC):
            nc.tensor.matmul(ps[:], lhsT=wo_sb[et][:, dt * 128:(dt + 1) * 128],
                             rhs=ONT[et][:], start=(et == 0), stop=(et == DC - 1),
                             skip_group_check=True)
        nc.vector.scalar_tensor_tensor(
            x2T[dt][:], ps[:], bout_sb[:, dt:dt + 1], y1T[dt][:],
            op0=ALU.add, op1=ALU.add)
        nc.scalar.copy(x2b[dt][:], x2T[dt][:])


def _phase_ln2(nc, tc, ctx, env):
    """LN2 stats + normalize into n2T (bf16)."""
    x2b, n2T = env["x2b"], env["n2T"]
    ones1, selbc1, eps5 = env["ones1"], env["selbc1"], env["eps5"]

    psq2 = ctx.enter_context(tc.tile_pool(name="e_sq", bufs=4))
    prow2 = ctx.enter_context(tc.tile_pool(name="e_rows", bufs=8))
    pst2 = ctx.enter_context(tc.tile_pool(name="e_st", bufs=2, space="PSUM"))
    pbc2 = ctx.enter_context(tc.tile_pool(name="e_bc", bufs=2, space="PSUM"))

    mu_ps = pst2.tile([1, NQ], F32, tag="st2", name="mu2_ps")
    msq_ps = pst2.tile([1, NQ], F32, tag="st2", name="msq2_ps")
    for dc in range(DC):
        nc.tensor.matmul(mu_ps[:], lhsT=ones1[:], rhs=x2b[dc][:],
                         start=(dc == 0), stop=(dc == DC - 1), skip_group_check=True)
    for dc in range(DC):
        sq = psq2.tile([128, NQ], BF, tag="sq2")
        if dc % 2 == 0:
            nc.vector.tensor_mul(sq[:], x2b[dc][:], x2b[dc][:])
        else:
            nc.scalar.activation(sq[:], x2b[dc][:], AF.Square)
        nc.tensor.matmul(msq_ps[:], lhsT=ones1[:], rhs=sq[:],
                         start=(dc == 0), stop=(dc == DC - 1), skip_group_check=True)

    invD = 1.0 / D
    muR = prow2.tile([1, NQ], BF, tag="row2", name="mu2R")
    nc.vector.tensor_scalar_mul(muR[:], mu_ps[:], invD)
    musq = prow2.tile([1, NQ], F32, tag="row2", name="musq2")
    nc.vector.tensor_mul(musq[:], muR[:], muR[:])
    var = prow2.tile([1, NQ], F32, tag="row2", name="var2")
    nc.vector.scalar_tensor_tensor(var[:], msq_ps[:], invD, musq[:],
                                   op0=ALU.mult, op1=ALU.subtract)
    nc.scalar.activation(var[:], var[:], AF.Sqrt, bias=eps5[0:1, 0:1])
    SR = prow2.tile([1, NQ], BF, tag="row2", name="S2R")
    nc.vector.reciprocal(SR[:], var[:])

    mu_b = pbc2.tile([128, NQ], F32, tag="bc2")
    nc.tensor.matmul(mu_b[:], lhsT=selbc1[:], rhs=muR[:], start=True, stop=True)
    S_b = pbc2.tile([128, NQ], F32, tag="bc2")
    nc.tensor.matmul(S_b[:], lhsT=selbc1[:], rhs=SR[:], start=True, stop=True)
    for dc in range(DC):
        tmp = psq2.tile([128, NQ], BF, tag="sq2")
        nc.vector.tensor_sub(tmp[:], x2b[dc][:], mu_b[:])
        nc.vector.tensor_mul(n2T[dc][:], tmp[:], S_b[:])


def _phase_ffn(nc, tc, ctx, env):
    """Interleaved FFN1(silu)/FFN2 with streamed w2 tiles."""
    w1_sb, w2T, cb1_sb = env["w1_sb"], env["w2T"], env["cb1_sb"]
    n2T = env["n2T"]

    pw2 = ctx.enter_context(tc.tile_pool(name="f_w2", bufs=4))
    pffs = ctx.enter_context(tc.tile_pool(name="f_ffs", bufs=3))
    pmmE = ctx.enter_context(tc.tile_pool(name="f_mm", bufs=2, space="PSUM"))
    ps_acc = env["ps_acc"]
    for ft in range(FT):
        w2t = pw2.tile([128, D], BF, tag="w2")
        nc.gpsimd.dma_start(w2t[:], w2T[ft * 128:(ft + 1) * 128, :])
        ps1 = pmmE.tile([128, NQ], F32, tag="mm1")
        for dc in range(DC):
            nc.tensor.matmul(ps1[:], lhsT=w1_sb[dc][:, ft * 128:(ft + 1) * 128],
                             rhs=n2T[dc][:], start=(dc == 0), stop=(dc == DC - 1),
                             skip_group_check=True)
        # silu(u) = u * sigmoid(u) with u = ps1 + cb1
        sig = pffs.tile([128, NQ], F32, tag="sig")
        nc.scalar.activation(sig[:], ps1[:], AF.Sigmoid, bias=cb1_sb[:, ft:ft + 1])
        ffs = pffs.tile([128, NQ], BF, tag="ffs")
        nc.vector.scalar_tensor_tensor(ffs[:], ps1[:], cb1_sb[:, ft:ft + 1], sig[:],
                                       op0=ALU.add, op1=ALU.mult)
        for dt in range(DC):
            nc.tensor.matmul(ps_acc[dt][:], lhsT=w2t[:, dt * 128:(dt + 1) * 128],
                             rhs=ffs[:], start=(ft == 0), stop=(ft == FT - 1),
                             skip_group_check=True)


def _phase_store(nc, tc, ctx, env):
    """Residual add, transpose feature-major result to token-major, store."""
    ps_acc, x2T, b2_sb = env["ps_acc"], env["x2T"], env["b2_sb"]
    identb, out = env["identb"], env["out"]
    pout = ctx.enter_context(tc.tile_pool(name="h_out", bufs=DC))
    posb = ctx.enter_context(tc.tile_pool(name="h_osb", bufs=2))
    ptr = ctx.enter_context(tc.tile_pool(name="h_tr", bufs=2, space="PSUM"))

    outT = []
    for dt in range(DC):
        t = pout.tile([128, NQ], BF, tag="outT", name=f"outT{dt}")
        nc.vector.scalar_tensor_tensor(
            t[:], ps_acc[dt][:], b2_sb[:, dt:dt + 1], x2T[dt][:],
            op0=ALU.add, op1=ALU.add)
        outT.append(t)
    QSZ = [128, 128, 128, 8]
    for qt in range(4):
        qsz = QSZ[qt]
        osb = posb.tile([128, D], F32, tag="osb")
        for dt in range(DC):
            tp = ptr.tile([128, 128], BF, tag="ptr")
            nc.tensor.transpose(tp[0:qsz, :],
                                outT[dt][:, qt * 128:qt * 128 + qsz], identb[:])
            nc.scalar.copy(osb[0:qsz, dt * 128:(dt + 1) * 128], tp[0:qsz, :])
        nc.sync.dma_start(out[qt * 128:qt * 128 + qsz, :], osb[0:qsz, :])


def build_program():
    nc = bacc.Bacc("TRN2")
    env = {}
    env["xT"] = nc.declare_dram_parameter("xT", [D, L], BF, isOutput=False)
    env["xqT"] = nc.declare_dram_parameter("xqT", [D, NQ], BF, isOutput=False)
    wqkvT = nc.declare_dram_parameter("wqkvT", [D, 3 * D], BF, isOutput=False)
    cbq = nc.declare_dram_parameter("cbq", [128, DC], F32, isOutput=False)
    woutT = nc.declare_dram_parameter("woutT", [D, D], BF, isOutput=False)
    bout = nc.declare_dram_parameter("bout", [128, DC], F32, isOutput=False)
    w1T = nc.declare_dram_parameter("w1T", [D, DFF], BF, isOutput=False)
    cb1 = nc.declare_dram_parameter("cb1", [128, FT], F32, isOutput=False)
    env["w2T"] = nc.declare_dram_parameter("w2T", [DFF, D], BF, isOutput=False)
    b2 = nc.declare_dram_parameter("b2", [128, DC], F32, isOutput=False)
    msk = nc.declare_dram_parameter("msk", [128, 3 * NJT], F32, isOutput=False)
    selc = nc.declare_dram_parameter("selc", [128, 5 * (NCH + 1)], BF, isOutput=False)
    selbcp = nc.declare_dram_parameter("selbcp", [5, 640], BF, isOutput=False)
    env["out"] = nc.declare_dram_parameter("out", [NQ, D], F32, isOutput=True)

    with tile.TileContext(nc) as tc, ExitStack() as top:
        top.enter_context(nc.allow_low_precision(
            reason="bf16 intermediates; rel-err gate is 2e-2"))
        pc = top.enter_context(tc.tile_pool(name="const", bufs=1))
        pw = top.enter_context(tc.tile_pool(name="weights", bufs=DC))
        pact = top.enter_context(tc.tile_pool(name="acts", bufs=DC))
        pmid = top.enter_context(ExitStack())

        pnt = pmid.enter_context(tc.tile_pool(name="ntp", bufs=DC))
        pnq = pmid.enter_context(tc.tile_pool(name="nqp", bufs=DC))
        env["nT"] = [pnt.tile([128, L], BF, tag="nt", name=f"nT{i}") for i in range(DC)]
        env["nqT"] = [pnq.tile([128, NQ], BF, tag="nq", name=f"nqT{i}") for i in range(DC)]
        # -- tiny constants first: the first stats matmul needs selc --
        selc_sb = pc.tile([128, 5 * (NCH + 1)], BF, tag="selc", name="selc_sb")
        nc.sync.dma_start(selc_sb[:], selc[:])
        env["sel5"] = [selc_sb[:, 5 * ci:5 * (ci + 1)] for ci in range(NCH + 1)]
        selbc = pc.tile([5, 640], BF, tag="selbc", name="selbc")
        nc.sync.dma_start(selbc[:], selbcp[:])
        env["selbc"] = selbc
        for name, prm, w in (("cbq_sb", cbq, DC), ("bout_sb", bout, DC),
                             ("b2_sb", b2, DC), ("cb1_sb", cb1, FT),
                             ("msk_sb", msk, 3 * NJT)):
            t = pc.tile([128, w], F32, tag=name, name=name)
            nc.sync.dma_start(t[:], prm[:])
            env[name] = t

        # input DMAs in 2 pieces per tile so stats matmuls chase the stream
        for half in range(2):
            for dc in range(DC):
                nc.sync.dma_start(
                    env["nT"][dc][:, half * 2 * CH:(half + 1) * 2 * CH],
                    env["xT"][dc * 128:(dc + 1) * 128, half * 2 * CH:(half + 1) * 2 * CH])
        for dc in range(DC):
            nc.sync.dma_start(env["nqT"][dc][:], env["xqT"][dc * 128:(dc + 1) * 128, :])

        # -- weight prefetch (big per-dc tiles) --
        env["wq_sb"] = []
        for dc in range(DC):
            t = pw.tile([128, 3 * D], BF, tag="wq", name=f"wq{dc}")
            nc.gpsimd.dma_start(t[:], wqkvT[dc * 128:(dc + 1) * 128, :])
            env["wq_sb"].append(t)
        env["wo_sb"] = []
        for dc in range(DC):
            t = pw.tile([128, D], BF, tag="wo", name=f"wo{dc}")
            nc.gpsimd.dma_start(t[:], woutT[dc * 128:(dc + 1) * 128, :])
            env["wo_sb"].append(t)
        env["w1_sb"] = []
        for dc in range(DC):
            t = pw.tile([128, DFF], BF, tag="w1", name=f"w1{dc}")
            nc.gpsimd.dma_start(t[:], w1T[dc * 128:(dc + 1) * 128, :])
            env["w1_sb"].append(t)

        selbc1 = pc.tile([1, 128], BF, tag="selbc1", name="selbc1")
        nc.vector.memset(selbc1[:], 1.0)
        env["selbc1"] = selbc1
        ones1 = pc.tile([128, 1], BF, tag="ones1", name="ones1")
        nc.vector.memset(ones1[:], 1.0)
        env["ones1"] = ones1
        eps5 = pc.tile([5, 1], F32, tag="eps5", name="eps5")
        nc.vector.memset(eps5[:], EPS)
        env["eps5"] = eps5
        ident = pc.tile([128, 128], F32, tag="ident", name="ident")
        make_identity(nc, ident[:])
        identb = pc.tile([128, 128], BF, tag="identb", name="identb")
        nc.vector.tensor_copy(identb[:], ident[:])
        env["identb"] = identb

        # -- persistent activations --
        env["y1T"] = [pact.tile([128, NQ], BF, tag="y1", name=f"y1T{i}") for i in range(DC)]
        env["x2T"] = [pact.tile([128, NQ], F32, tag="x2", name=f"x2T{i}") for i in range(DC)]
        env["x2b"] = [pact.tile([128, NQ], BF, tag="x2b", name=f"x2b{i}") for i in range(DC)]
        env["n2T"] = [pact.tile([128, NQ], BF, tag="n2", name=f"n2T{i}") for i in range(DC)]

        with ExitStack() as mid:
            pkt = mid.enter_context(tc.tile_pool(name="ktp", bufs=DC))
            pqt = mid.enter_context(tc.tile_pool(name="qtp", bufs=DC))
            pva = mid.enter_context(tc.tile_pool(name="vap", bufs=NJT))
            pont = mid.enter_context(tc.tile_pool(name="ontp", bufs=DC))
            env["KT"] = [pkt.tile([128, L], BF, tag="kt", name=f"KT{i}") for i in range(DC)]
            env["QT"] = [pqt.tile([128, NQ], BF, tag="qt", name=f"QT{i}") for i in range(DC)]
            env["VA"] = [pva.tile([128, H * 65], BF, tag="va", name=f"VA{i}") for i in range(NJT)]
            env["ONT"] = [pont.tile([128, NQ], BF, tag="ont", name=f"ONT{i}") for i in range(DC)]

            with ExitStack() as ctx:
                _phase_ln1(nc, tc, ctx, env)
            with ExitStack() as ctx:
                _phase_v(nc, tc, ctx, env)
            with ExitStack() as ctx:
                _phase_attn(nc, tc, ctx, env)
            with ExitStack() as ctx:
                _phase_outproj(nc, tc, ctx, env)
        pmid.close()

        with ExitStack() as ctx:
            _phase_ln2(nc, tc, ctx, env)
        with ExitStack() as ctx:
            pacc = ctx.enter_context(tc.tile_pool(name="f_acc", bufs=DC, space="PSUM"))
            env["ps_acc"] = [pacc.tile([128, NQ], F32, tag="acc", name=f"ps_acc{i}")
                             for i in range(DC)]
            with ExitStack() as ctx2:
                _phase_ffn(nc, tc, ctx2, env)
            with ExitStack() as ctx2:
                _phase_store(nc, tc, ctx2, env)

    nc.finalize()
    return nc


_NC = None


def _get_nc():
    global _NC
    if _NC is None:
        _NC = build_program()
    return _NC


def _host_prepare(inputs):
    """Fold constants and lay out per-core input maps."""
    import ml_dtypes
    bf16 = ml_dtypes.bfloat16
    f32 = np.float32
    x = np.asarray(inputs["x"], f32)
    memory = np.asarray(inputs["memory"], f32)
    w_qkv = np.asarray(inputs["w_qkv"], f32)
    w_out = np.asarray(inputs["w_out"], f32)
    b_out = np.asarray(inputs["b_out"], f32)
    g_att = np.asarray(inputs["ln_att_g"], f32)
    b_att = np.asarray(inputs["ln_att_b"], f32)
    g2 = np.asarray(inputs["ln2_g"], f32)
    bb2 = np.asarray(inputs["ln2_b"], f32)
    w1 = np.asarray(inputs["w1"], f32)
    b1 = np.asarray(inputs["b1"], f32)
    w2 = np.asarray(inputs["w2"], f32)
    b2v = np.asarray(inputs["b2"], f32)

    qscale = f32(DH ** -0.5)
    w_qkv_eff = w_qkv * g_att[None, :]
    w_qkv_eff[:D] *= qscale
    cb_qkv = w_qkv @ b_att
    cb_q = (cb_qkv[:D] * qscale).astype(f32)
    cb_v = cb_qkv[2 * D:].astype(f32)
    b_out_eff = (b_out + w_out @ cb_v).astype(f32)
    w1_eff = w1 * g2[None, :]
    cb1_eff = (w1 @ bb2 + b1).astype(f32)

    def cols(v):
        # [N] vector -> [128, N//128] per-partition bias layout
        return np.ascontiguousarray(v.reshape(-1, 128).T)

    selc = np.zeros((128, 5 * (NCH + 1)), np.float32)
    for ci in range(NCH + 1):
        selc[:, 5 * ci + ci] = 1.0
    selbcp = np.zeros((5, 640), np.float32)
    for k in range(5):
        selbcp[k, k * 128:(k + 1) * 128] = 1.0
    shared = {
        "selc": selc.astype(bf16),
        "selbcp": selbcp.astype(bf16),
        "wqkvT": np.ascontiguousarray(w_qkv_eff.T).astype(bf16),
        "cbq": cols(cb_q),
        "woutT": np.ascontiguousarray(w_out.T).astype(bf16),
        "bout": cols(b_out_eff),
        "w1T": np.ascontiguousarray(w1_eff.T).astype(bf16),
        "cb1": cols(cb1_eff),
        "w2T": np.ascontiguousarray(w2.T).astype(bf16),
        "b2": cols(b2v),
    }

    in_maps = []
    for c in range(NCORES):
        b, hf = divmod(c, 2)
        x_aug = np.concatenate([memory[b, :T], x[b]], axis=0)      # [L, D]
        q0 = T + hf * NQ
        LcA = (5 + 2 * hf) * NPATCH
        LcB = (6 + 2 * hf) * NPATCH
        j = np.arange(NJT * 128)
        sa = (j < LcB).astype(f32)
        ba = np.where(sa > 0, 0.0, -30.0).astype(f32)
        ma = (j < LcA).astype(f32)
        mskv = np.concatenate(
            [v.reshape(NJT, 128).T for v in (sa, ba, ma)], axis=1)
        in_maps.append({
            "xT": np.ascontiguousarray(x_aug.T).astype(bf16),
            "xqT": np.ascontiguousarray(x_aug[q0:q0 + NQ].T).astype(bf16),
            "msk": np.ascontiguousarray(mskv),
            **shared,
        })
    return in_maps


def _assemble(results):
    out = np.zeros((B, T, D), np.float32)
    for c in range(NCORES):
        b, hf = divmod(c, 2)
        out[b, hf * NQ:(hf + 1) * NQ, :] = results[c]["out"]
    return out


def kernel(**inputs):
    nc = _get_nc()
    in_maps = _host_prepare(inputs)
    res = run_bass_kernel_spmd(nc, in_maps, list(range(NCORES)))
    return _assemble(res.results)


def _ensure_ntff_hook():
    """Provide antenv.axon_hooks (absent in this image) so trace=True can
    drive NTFF capture through libaxon_pjrt.so, mirroring trn_boot.py."""
    import contextlib
    import ctypes
    import types

    try:
        from antenv.axon_hooks import get_axon_ntff_profile_hook  # noqa: F401
        return
    except ImportError:
        pass
    import antenv

    so_path = "/opt/axon/libaxon_pjrt.so"
    lib = ctypes.CDLL(so_path)
    if not hasattr(lib, "axon_start_nrt_profile"):
        raise RuntimeError("libaxon_pjrt.so lacks NTFF profile symbols")
    lib.axon_start_nrt_profile.argtypes = [ctypes.POINTER(ctypes.c_int64),
                                           ctypes.c_size_t]
    lib.axon_start_nrt_profile.restype = ctypes.c_int64
    lib.axon_stop_nrt_profile.argtypes = [ctypes.c_char_p]
    lib.axon_stop_nrt_profile.restype = ctypes.c_int64

    @contextlib.contextmanager
    def _hook(output_dir, device_ids):
        import jax
        jax.devices()
        if device_ids:
            ids = (ctypes.c_int64 * len(device_ids))(*device_ids)
            rc = lib.axon_start_nrt_profile(ids, len(device_ids))
        else:
            rc = lib.axon_start_nrt_profile(None, 0)
        if rc != 0:
            raise RuntimeError(f"axon_start_nrt_profile rc={rc}")
        try:
            yield
        finally:
            n = lib.axon_stop_nrt_profile(str(output_dir).encode())
            print(f"ntff profile: {n} file(s) written to {output_dir}",
                  file=sys.stderr)

    box = {"h": _hook}
    mod = types.ModuleType("antenv.axon_hooks")
    mod.set_axon_ntff_profile_hook = lambda h: box.__setitem__("h", h)
    mod.get_axon_ntff_profile_hook = lambda: box["h"]
    sys.modules["antenv.axon_hooks"] = mod
    antenv.axon_hooks = mod


def kernel_traced(**inputs):
    """Like kernel() but with NTFF profiling; returns (out, exec_time_ns)."""
    import tempfile

    from concourse import bass_utils as _bu
    _ensure_ntff_hook()
    _bu.upload_artifacts = lambda tmpdir: f"local:{tmpdir}"  # no bucket creds here
    nc = _get_nc()
    in_maps = _host_prepare(inputs)
    tmpdir = tempfile.mkdtemp(prefix="ntff_")
    res = run_bass_kernel_spmd(nc, in_maps, list(range(NCORES)), trace=True,
                               tmpdir=tmpdir)
    return _assemble(res.results), res.exec_time_ns
